# revision 49
# baseline (speedup 1.0000x reference)
"""Multi-head attention on 8 TRN2 NeuronCores.

Problem: x[2, 2048, 1024], w_qkv[1024, 3072], w_out[1024, 1024] (f32).
  qkv = x @ w_qkv; q,k,v per 16 heads of dim 64; softmax(q k^T / 8) v; out proj.

Sharding: 16 heads split 8 ways (one head-PAIR per core, both batches on
every core).  Output ownership is interleaved by batch: core c owns rows
(b=0, cols c*256:(c+1)*256) and (b=1, same cols), so that after each
(head, batch) attention unit a small AllToAll of [8, 64, 256] can fire in
which EVERY slot carries final data.  Four such exchanges; only the last
is exposed, and it overlaps the b=0 half of the output projection.

The kernel is scheduled around the Scalar engine (exp is 140us of the
~190us total work): the k-projection for batch 0 is emitted first so the
first scores matmul (and exp) issues at ~14us, and every other piece of
PE work (q cols, v tiles, batch-1 qkv, broadcast matmuls) is woven
between score/attend matmuls so the exp stream never stalls on a block
of projection work.

Layout: scores are computed TRANSPOSED (S^T[m, l] tiles); softmax sum
over the partition axis comes free from a ones-column appended to v in
the attn@v matmul (out rows = [o^T; colsums]).  exp() runs on ScalarE
straight out of PSUM with the 1/8 scale folded in.  Normalization:
recip(colsums) -> bf16 row, a ones-row matmul broadcasts it across 64
partitions, one DVE multiply writes normalized o^T bf16 -- no transposes.

q is stored STACKED ([q_h0; q_h1] on 128 partitions, no zero pad); the
per-head k tiles are zero-padded in complementary halves (k_h0 on rows
0:64, k_h1 on rows 64:128) so each scores matmul contracts over the full
128 partitions (keeps the HAM clock gate at 2.4 GHz) while the other
head's q rows are killed by zero weights.

Compute dtype bf16 (f32 accumulation in PSUM).
"""

import sys
import types

sys.path.insert(0, "/opt/trn_rl_repo")

import numpy as np
import ml_dtypes

import concourse.bass as bass
import concourse.mybir as mybir
import concourse.tile as tile
from concourse import bacc
from concourse import bass_utils
from concourse.masks import make_identity

# If the image's antenv lacks the axon_hooks module, run_bass_kernel_spmd's
# trace path (reachable via BASS_TRACE=1) would die on import.  Provide the
# registry so tracing degrades gracefully instead (hook stays None unless
# trn_boot registered one).
try:
    import antenv.axon_hooks  # noqa: F401
except ImportError:
    _hooks = types.ModuleType("antenv.axon_hooks")
    _hooks._hook = None
    _hooks.set_axon_ntff_profile_hook = (
        lambda h: setattr(_hooks, "_hook", h))
    _hooks.get_axon_ntff_profile_hook = lambda: _hooks._hook
    sys.modules["antenv.axon_hooks"] = _hooks

# Artifact upload needs bucket credentials; fall back to the local dir so a
# traced run in a sandboxed container still completes.
_orig_upload = bass_utils.upload_artifacts


def _safe_upload(tmpdir):
    try:
        return _orig_upload(tmpdir)
    except Exception:
        return tmpdir


bass_utils.upload_artifacts = _safe_upload

B, L, D, H, DH = 2, 2048, 1024, 16, 64
BL = B * L  # 4096
SCALE = DH ** -0.5
N_CORES = 8
BF16 = mybir.dt.bfloat16
F32 = mybir.dt.float32
Exp = mybir.ActivationFunctionType.Exp

KT = D // 128          # 8 k-tiles over the model dim
MT = L // 128          # 16 m-tiles per batch
LC = L // 512          # 4 l-chunks of 512 per batch
VT = BL // 128         # 32 v row-tiles over (b, l)
# units in emission order: (hl, b)
UNITS = [(0, 0), (1, 0), (0, 1), (1, 1)]


def _build():
    nc = bacc.Bacc("TRN2", target_bir_lowering=False, debug=False,
                   num_devices=N_CORES)
    xT_ext = nc.declare_dram_parameter("xT", [D, BL], BF16, isOutput=False)
    # q (128) | k (128) | v (128) columns for this core's head pair
    wqkv_ext = nc.declare_dram_parameter("wqkv", [D, 384], BF16,
                                         isOutput=False)
    wout_ext = nc.declare_dram_parameter("wout", [D, D], BF16, isOutput=False)
    out_ext = nc.declare_dram_parameter("out", [512, D], BF16, isOutput=True)

    with tile.TileContext(nc) as tc:
        with (
            tc.tile_pool(name="big", bufs=1) as big,
            tc.tile_pool(name="pt", bufs=24) as ptp,
            tc.tile_pool(name="small", bufs=3) as small,
            tc.tile_pool(name="psum_st", bufs=2, space="PSUM") as pst,
            tc.tile_pool(name="psum_ov", bufs=2, space="PSUM") as pov,
            tc.tile_pool(name="psum_bc", bufs=2, space="PSUM") as pbc,
            tc.tile_pool(name="dram", bufs=1, space="DRAM") as dram,
        ):
            # ---- static SBUF tensors ----
            xT_t = [big.tile([128, BL], BF16, tag=f"xT{k}", name=f"xT{k}")
                    for k in range(KT)]
            wqkv_t = [big.tile([128, 384], BF16, tag=f"wqkv{k}",
                               name=f"wqkv{k}") for k in range(KT)]
            # DMA priority: the 8 wqkv tiles land on queues 0-7 while the
            # first xT b0 column-chunks land on 8-15 in parallel; b0 moves
            # in fine [128, 512] chunks (lowest latency to first scores),
            # b1 in coarser [128, 2048] chunks (fewer descriptors).
            # row-split the latency-critical first loads: a DMA's latency is
            # ~46ns x (descriptor = partition-row) serial per queue, so two
            # 64-row DMAs on two queues halve time-to-ready.
            for k in range(KT):
                for h in range(2):
                    rs = slice(k * 128 + h * 64, k * 128 + (h + 1) * 64)
                    nc.sync.dma_start(wqkv_t[k][h * 64:(h + 1) * 64, :],
                                      wqkv_ext[rs, :])
            for k in range(KT):
                for h in range(2):
                    rs = slice(k * 128 + h * 64, k * 128 + (h + 1) * 64)
                    nc.sync.dma_start(
                        xT_t[k][h * 64:(h + 1) * 64, 0:512],
                        xT_ext[rs, 0:512])
            for cc in range(1, 4):
                for k in range(KT):
                    cs = slice(cc * 512, (cc + 1) * 512)
                    nc.sync.dma_start(
                        xT_t[k][:, cs],
                        xT_ext[k * 128:(k + 1) * 128, cs])
            for k in range(KT):
                nc.sync.dma_start(
                    xT_t[k][:, 2048:4096],
                    xT_ext[k * 128:(k + 1) * 128, 2048:4096])

            ident_b = big.tile([128, 128], BF16, tag="ident_b")
            make_identity(nc, ident_b[:])

            # Preload the Exp activation table while DMAs stream in, so the
            # first real exp doesn't pay the ~1.3us table load.  Emitted
            # before the other Pool-engine memsets so it lands early.
            dum = big.tile([1, 16], F32, tag="dum")
            nc.gpsimd.memset(dum[:], 0.0)
            dum2 = big.tile([1, 16], BF16, tag="dum2")
            nc.scalar.activation(dum2[:], dum[:], Exp, scale=1.0)

            # q stacked (h0 rows 0:64, h1 rows 64:128) -- no padding.
            qs_t = big.tile([128, BL], BF16, tag="qs", name="qs")
            # k per head, zero-padded in complementary halves.
            kp_t = [big.tile([128, BL], BF16, tag=f"kp{h}", name=f"kp{h}")
                    for h in range(2)]
            nc.vector.memset(kp_t[0][64:128, :], 0.0)
            nc.vector.memset(kp_t[1][0:64, :], 0.0)
            # v: cols [h*128 : h*128+64] = head h, +64 = ones, rest zero
            v_t = [big.tile([128, 256], BF16, tag=f"v{t}", name=f"v{t}")
                   for t in range(VT)]
            for t in range(VT):
                vv = v_t[t][:].rearrange("p (h c) -> p h c", h=2)
                nc.gpsimd.memset(vv[:, :, 64:65], 1.0)
                nc.gpsimd.memset(vv[:, :, 65:128], 0.0)
            # final o^T for our 2 heads, all 4096 cols
            oT_f = big.tile([128, BL], BF16, tag="oT")
            # received heads for our 512 output rows (cols 0:256 b0, 256:512 b1)
            ogT_t = [big.tile([128, 512], BF16, tag=f"ogT{k}", name=f"ogT{k}")
                     for k in range(KT)]
            # softmax-denominator staging: denom row copied out of PSUM
            # (reciprocal_approx_fast misreads PSUM inputs), its reciprocal,
            # and the partition-broadcast copy; double-buffered by lc parity.
            sdf = [big.tile([1, 512], F32, tag=f"sdf{i}", name=f"sdf{i}")
                   for i in range(2)]
            scrf = [big.tile([1, 512], F32, tag=f"scrf{i}", name=f"scrf{i}")
                    for i in range(2)]
            bcsf = [big.tile([64, 512], F32, tag=f"bcsf{i}", name=f"bcsf{i}")
                    for i in range(2)]

            # Warm the HAM clock gate during the initial DMA wait: ~30
            # back-to-back matmuls lift the PE to 2.4 GHz before the first
            # real matmul issues.  Output is scratch nobody reads.
            warm = pov.tile([128, 512], F32, tag="ov", name="warm")
            for i in range(30):
                nc.tensor.matmul(warm[:, 0:128], ident_b[:], ident_b[:],
                                 start=(i == 0), stop=(i == 29))

            # ---- background PE work generators (woven between scores) ----
            qk_pending = {}  # (m, ncol) -> psum tile with k 0..3 accumulated

            def emit_qk_half(m, ncol, half):
                # m: 0 -> q, 1 -> k.  One 512-col chunk of the projection,
                # split into two 4-matmul half-chains (the PSUM accumulation
                # pauses in between) so background pops stay fine-grained.
                if half == 0:
                    ps = pbc.tile([128, 512], F32, tag="bc",
                                  name=f"qk{m}_{ncol}")
                    qk_pending[(m, ncol)] = ps
                else:
                    ps = qk_pending.pop((m, ncol))
                for k in range(half * 4, half * 4 + 4):
                    nc.tensor.matmul(
                        ps[:],
                        wqkv_t[k][:, m * 128:(m + 1) * 128],
                        xT_t[k][:, ncol * 512:(ncol + 1) * 512],
                        start=(k == 0), stop=(k == KT - 1),
                    )
                if half == 0:
                    return
                cs = slice(ncol * 512, (ncol + 1) * 512)
                if m == 0:
                    nc.vector.tensor_copy(qs_t[:, cs], ps[:])
                else:
                    nc.vector.tensor_copy(kp_t[0][0:64, cs], ps[0:64, :])
                    nc.vector.tensor_copy(kp_t[1][64:128, cs],
                                          ps[64:128, :])

            def emit_qk_col(m, ncol):
                emit_qk_half(m, ncol, 0)
                emit_qk_half(m, ncol, 1)

            def emit_v_tile(t):
                ps = pbc.tile([128, 128], F32, tag="bc", name=f"v_ps{t}")
                for k in range(KT):
                    nc.tensor.matmul(
                        ps[:],
                        xT_t[k][:, t * 128:(t + 1) * 128],
                        wqkv_t[k][:, 256:384],
                        start=(k == 0), stop=(k == KT - 1),
                    )
                vv = v_t[t][:].rearrange("p (h c) -> p h c", h=2)
                nc.vector.tensor_copy(
                    vv[:, :, 0:64],
                    ps[:].rearrange("p (h c) -> p h c", h=2))

            def make_bg():
                # Generator of small PE work chunks, popped between
                # foreground score/attend matmuls.  Ordering constraints:
                # k ncol N before scores (unit 0, lc 0, mp >= 2N); q ncol N
                # before scores of (unit 0, lc N); v tile t of b0 before the
                # attend chunk reading it (block 2, pops precede attend
                # chunks within a slot); qk b1 before unit 2 scores; v b1
                # before unit 2 attends.
                def qk(m, ncol):
                    yield ('qkh', m, ncol, 0)
                    yield ('qkh', m, ncol, 1)

                for ncol in range(1, LC):
                    yield from qk(1, ncol)
                yield from qk(0, 1)
                for t in range(4, VT // 4):
                    yield ('v', t)
                yield from qk(0, 2)
                for t in range(VT // 4, VT // 2):
                    yield ('v', t)
                yield from qk(0, 3)
                for ncol in range(LC, 2 * LC):
                    yield from qk(1, ncol)
                    yield from qk(0, ncol)
                for t in range(VT // 2, VT):
                    yield ('v', t)

            bg = make_bg()
            bg_done = [False]
            bg_deficit = [0.0]  # us of background work we owe the stream
            emitted = {('qk', 1, 0), ('qk', 0, 0)}  # prologue

            def bg_step():
                item = next(bg, None)
                if item is None:
                    bg_done[0] = True
                    return False
                if item[0] == 'v':
                    emitted.add(item)
                    emit_v_tile(item[1])
                    bg_deficit[0] -= 0.55
                else:
                    _, m, ncol, half = item
                    emit_qk_half(m, ncol, half)
                    if half == 1:
                        emitted.add(('qk', m, ncol))
                    bg_deficit[0] -= 0.95
                return True

            def bg_pop(budget_us):
                # emit background chunks worth ~budget_us of PE time
                bg_deficit[0] += budget_us
                while not bg_done[0] and bg_deficit[0] > 0:
                    if not bg_step():
                        return

            def bg_need(item):
                # structural deadline: force-drain the queue until `item`
                # has been emitted (it must precede its first consumer in
                # the in-order PE queue, or the kernel deadlocks).
                while item not in emitted and not bg_done[0]:
                    bg_step()

            # ---- attention unit pieces ----
            # one exchange per batch, fired after that batch's second
            # (hl=1) unit, carrying both head rows: [8, 128, 256]
            cc_in = [dram.tile([N_CORES, 128, 256], BF16, name=f"cc_in{i}")
                     for i in range(2)]
            cc_out = [dram.tile([N_CORES, 128, 256], BF16, name=f"cc_out{i}")
                      for i in range(2)]

            def emit_scores(hl, b, lc, mp):
                # one st tile = S^T for m-tiles (2mp, 2mp+1) x 512 l-cols
                ls = slice(b * L + lc * 512, b * L + (lc + 1) * 512)
                st = pst.tile([128, 1024], F32, tag="st")
                for h2 in range(2):
                    mt = 2 * mp + h2
                    nc.tensor.matmul(
                        st[:, h2 * 512:(h2 + 1) * 512],
                        kp_t[hl][:, b * L + mt * 128:b * L + (mt + 1) * 128],
                        qs_t[:, ls],
                        start=True, stop=True,
                    )
                pt = ptp.tile([128, 2, 512], BF16, tag="pt")
                nc.scalar.activation(pt[:], st[:], Exp, scale=SCALE)
                return pt

            def emit_av_chunk(hl, b, ov, pts, mt0, mt1):
                for mt in range(mt0, mt1):
                    nc.tensor.matmul(
                        ov[:],
                        v_t[b * MT + mt][:, hl * 128:(hl + 1) * 128],
                        pts[mt // 2][:, mt % 2, :],
                        start=(mt == 0), stop=(mt == MT - 1),
                    )

            def emit_norm(u, hl, b, lc, ov):
                # normalize ov rows 0:64 by its colsum row (64) and write
                # bf16 o^T.  The reciprocal row is broadcast across
                # partitions on the idle GpSimd engine, keeping the PE
                # queue out of the chain.  After the second (hl=1) unit of
                # a batch, both head rows of this window are final -- stage
                # the two 256-col a2a slots.
                sd = sdf[lc % 2]
                sf = scrf[lc % 2]
                bf = bcsf[lc % 2]
                nc.vector.tensor_copy(sd[0:1, :], ov[64:65, :])
                nc.vector.reciprocal_approx_fast(sf[0:1, :], sd[0:1, :])
                nc.gpsimd.partition_broadcast(bf[:], sf[0:1, :])
                hs = slice(hl * 64, (hl + 1) * 64)
                win = slice(b * L + lc * 512, b * L + (lc + 1) * 512)
                nc.vector.tensor_mul(oT_f[hs, win], ov[0:64, :], bf[:])
                if hl == 1:
                    for jj in range(2):
                        j = 2 * lc + jj
                        nc.sync.dma_start(
                            cc_in[b][j],
                            oT_f[:, b * L + j * 256:b * L + (j + 1) * 256])

            def emit_exchange(b):
                nc.gpsimd.collective_compute(
                    "AllToAll",
                    mybir.AluOpType.bypass,
                    ins=[cc_in[b].opt()],
                    outs=[cc_out[b].opt()],
                    replica_groups=[list(range(N_CORES))],
                )
                for k in range(N_CORES):
                    for h in range(2):
                        nc.sync.dma_start(
                            ogT_t[k][h * 64:(h + 1) * 64,
                                     b * 256:(b + 1) * 256],
                            cc_out[b][k][h * 64:(h + 1) * 64, :])

            # ---- emission schedule ----
            # Prologue: only k and q for ncol 0 -- the first scores matmul
            # needs just those, so the exp stream starts ~10us in; the
            # remaining k/q columns, all v tiles and batch-1 qkv are woven
            # between foreground matmuls via the background queue, each
            # ahead of its first consumer.
            emit_qk_col(1, 0)
            emit_qk_col(0, 0)
            for t in range(4):  # v tiles of cc0 -- same DMA dep, PE idle
                emitted.add(('v', t))
                emit_v_tile(t)

            # Blocks: one per (unit, lc).  Block i emits scores for chunk i
            # and (woven between them) the attend chain for chunk i-2 --
            # the lag keeps the PE queue from deadlocking on pt buffers and
            # gives the background queue room in the first two blocks.
            chunks = [(u, hl, b, lc)
                      for u, (hl, b) in enumerate(UNITS) for lc in range(LC)]
            pts_of = {}
            n_chunks = len(chunks)
            AVLAG = 2

            for i in range(n_chunks + AVLAG):
                sc = chunks[i] if i < n_chunks else None
                avi = i - AVLAG
                av = chunks[avi] if avi >= 0 else None
                if av is not None:
                    au, ahl, ab, alc = av
                    aov = pov.tile([128, 512], F32, tag="ov")
                    apts = pts_of.pop(avi)
                if sc is not None:
                    su, shl, sb, slc = sc
                    bg_need(('qk', 0, sb * LC + slc))  # q cols of this lc
                    pts = []
                    for mp in range(MT // 2):
                        bg_need(('qk', 1, sb * LC + mp // 2))  # k m-tiles
                        pts.append(emit_scores(shl, sb, slc, mp))
                        if av is not None:
                            bg_pop(0.5)
                            bg_need(('v', ab * MT + 2 * mp + 1))
                            emit_av_chunk(ahl, ab, aov, apts,
                                          2 * mp, 2 * mp + 2)
                        else:
                            bg_pop(1.0)
                    pts_of[i] = pts
                else:
                    # tail blocks: attend only (exp stream is draining)
                    bg_need(('v', ab * MT + MT - 1))
                    emit_av_chunk(ahl, ab, aov, apts, 0, MT)
                if av is not None:
                    emit_norm(au, ahl, ab, alc, aov)
                    if alc == LC - 1 and ahl == 1:
                        emit_exchange(ab)
            bg_pop(100)  # safety: drain any background leftovers

            # ---- output projection for our 512 rows ----
            # lt 0,1 = b0 (gated on exchanges 0,1 -- long arrived, runs
            # during the final exchange); lt 2,3 = b1 (gated on exchange 3).
            wout_t = [big.tile([128, D], BF16, tag=f"xT{k}", name=f"wout{k}")
                      for k in range(KT)]
            for k in range(KT):
                nc.sync.dma_start(wout_t[k][:],
                                  wout_ext[k * 128:(k + 1) * 128, :])
            for lt in range(4):
                for nt in range(2):
                    ps = pst.tile([128, 1024], F32, tag="st")
                    for k in range(KT):
                        nc.tensor.matmul(
                            ps[:, 0:512],
                            ogT_t[k][:, lt * 128:(lt + 1) * 128],
                            wout_t[k][:, nt * 512:(nt + 1) * 512],
                            start=(k == 0), stop=(k == KT - 1),
                        )
                    osb = small.tile([128, 512], BF16, tag="osb")
                    # alternate copy engines: ScalarE is idle at the tail
                    if nt == 0:
                        nc.vector.tensor_copy(osb[:], ps[:, 0:512])
                    else:
                        nc.scalar.copy(osb[:], ps[:, 0:512])
                    for h in range(2):
                        nc.sync.dma_start(
                            out_ext[lt * 128 + h * 64:lt * 128 + (h + 1) * 64,
                                    nt * 512:(nt + 1) * 512],
                            osb[h * 64:(h + 1) * 64, :])

    nc.compile()
    return nc


_NC_CACHE = None


def _get_nc():
    global _NC_CACHE
    if _NC_CACHE is None:
        _NC_CACHE = _build()
    return _NC_CACHE


def _make_in_maps(x, w_qkv, w_out):
    x = np.asarray(x, dtype=np.float32)
    w_qkv = np.asarray(w_qkv, dtype=np.float32)
    w_out = np.asarray(w_out, dtype=np.float32)
    bf = ml_dtypes.bfloat16
    xT = np.ascontiguousarray(
        x.transpose(2, 0, 1).reshape(D, BL)).astype(bf)
    wout_b = w_out.astype(bf)
    in_maps = []
    for c in range(N_CORES):
        cs = slice(c * 128, (c + 1) * 128)
        wqkv_c = np.ascontiguousarray(
            np.concatenate([w_qkv[:, cs], w_qkv[:, D:][:, cs],
                            w_qkv[:, 2 * D:][:, cs]], axis=1)
        ).astype(bf)
        in_maps.append({"xT": xT, "wqkv": wqkv_c, "wout": wout_b})
    return in_maps


def _run(x, w_qkv, w_out, trace=False):
    nc = _get_nc()
    in_maps = _make_in_maps(x, w_qkv, w_out)
    res = bass_utils.run_bass_kernel_spmd(
        nc, in_maps, list(range(N_CORES)), trace=trace)
    out = np.empty((B, L, D), dtype=np.float32)
    for c in range(N_CORES):
        r = np.asarray(res.results[c]["out"]).astype(np.float32)
        out[0, c * 256:(c + 1) * 256, :] = r[0:256]
        out[1, c * 256:(c + 1) * 256, :] = r[256:512]
    return out, res


def kernel(x, w_qkv, w_out):
    out, _ = _run(x, w_qkv, w_out, trace=False)
    return out


# revision 50
# speedup vs baseline: 1.0047x; 1.0047x over previous
"""Multi-head attention on 8 TRN2 NeuronCores.

Problem: x[2, 2048, 1024], w_qkv[1024, 3072], w_out[1024, 1024] (f32).
  qkv = x @ w_qkv; q,k,v per 16 heads of dim 64; softmax(q k^T / 8) v; out proj.

Sharding: 16 heads split 8 ways (one head-PAIR per core, both batches on
every core).  Output ownership is interleaved by batch: core c owns rows
(b=0, cols c*256:(c+1)*256) and (b=1, same cols), so that after each
(head, batch) attention unit a small AllToAll of [8, 64, 256] can fire in
which EVERY slot carries final data.  Four such exchanges; only the last
is exposed, and it overlaps the b=0 half of the output projection.

The kernel is scheduled around the Scalar engine (exp is 140us of the
~190us total work): the k-projection for batch 0 is emitted first so the
first scores matmul (and exp) issues at ~14us, and every other piece of
PE work (q cols, v tiles, batch-1 qkv, broadcast matmuls) is woven
between score/attend matmuls so the exp stream never stalls on a block
of projection work.

Layout: scores are computed TRANSPOSED (S^T[m, l] tiles); softmax sum
over the partition axis comes free from a ones-column appended to v in
the attn@v matmul (out rows = [o^T; colsums]).  exp() runs on ScalarE
straight out of PSUM with the 1/8 scale folded in.  Normalization:
recip(colsums) -> bf16 row, a ones-row matmul broadcasts it across 64
partitions, one DVE multiply writes normalized o^T bf16 -- no transposes.

q is stored STACKED ([q_h0; q_h1] on 128 partitions, no zero pad); the
per-head k tiles are zero-padded in complementary halves (k_h0 on rows
0:64, k_h1 on rows 64:128) so each scores matmul contracts over the full
128 partitions (keeps the HAM clock gate at 2.4 GHz) while the other
head's q rows are killed by zero weights.

Compute dtype bf16 (f32 accumulation in PSUM).
"""

import sys
import types

sys.path.insert(0, "/opt/trn_rl_repo")

import numpy as np
import ml_dtypes

import concourse.bass as bass
import concourse.mybir as mybir
import concourse.tile as tile
from concourse import bacc
from concourse import bass_utils
from concourse.masks import make_identity

# If the image's antenv lacks the axon_hooks module, run_bass_kernel_spmd's
# trace path (reachable via BASS_TRACE=1) would die on import.  Provide the
# registry so tracing degrades gracefully instead (hook stays None unless
# trn_boot registered one).
try:
    import antenv.axon_hooks  # noqa: F401
except ImportError:
    _hooks = types.ModuleType("antenv.axon_hooks")
    _hooks._hook = None
    _hooks.set_axon_ntff_profile_hook = (
        lambda h: setattr(_hooks, "_hook", h))
    _hooks.get_axon_ntff_profile_hook = lambda: _hooks._hook
    sys.modules["antenv.axon_hooks"] = _hooks

# Artifact upload needs bucket credentials; fall back to the local dir so a
# traced run in a sandboxed container still completes.
_orig_upload = bass_utils.upload_artifacts


def _safe_upload(tmpdir):
    try:
        return _orig_upload(tmpdir)
    except Exception:
        return tmpdir


bass_utils.upload_artifacts = _safe_upload

B, L, D, H, DH = 2, 2048, 1024, 16, 64
BL = B * L  # 4096
SCALE = DH ** -0.5
N_CORES = 8
BF16 = mybir.dt.bfloat16
F32 = mybir.dt.float32
Exp = mybir.ActivationFunctionType.Exp

KT = D // 128          # 8 k-tiles over the model dim
MT = L // 128          # 16 m-tiles per batch
LC = L // 512          # 4 l-chunks of 512 per batch
VT = BL // 128         # 32 v row-tiles over (b, l)
# units in emission order: (hl, b)
UNITS = [(0, 0), (1, 0), (0, 1), (1, 1)]


def _build():
    nc = bacc.Bacc("TRN2", target_bir_lowering=False, debug=False,
                   num_devices=N_CORES)
    xT_ext = nc.declare_dram_parameter("xT", [D, BL], BF16, isOutput=False)
    # q (128) | k (128) | v (128) columns for this core's head pair
    wqkv_ext = nc.declare_dram_parameter("wqkv", [D, 384], BF16,
                                         isOutput=False)
    wout_ext = nc.declare_dram_parameter("wout", [D, D], BF16, isOutput=False)
    out_ext = nc.declare_dram_parameter("out", [512, D], BF16, isOutput=True)

    with tile.TileContext(nc) as tc:
        with (
            tc.tile_pool(name="big", bufs=1) as big,
            tc.tile_pool(name="pt", bufs=24) as ptp,
            tc.tile_pool(name="small", bufs=3) as small,
            tc.tile_pool(name="psum_st", bufs=2, space="PSUM") as pst,
            tc.tile_pool(name="psum_ov", bufs=2, space="PSUM") as pov,
            tc.tile_pool(name="psum_bc", bufs=2, space="PSUM") as pbc,
            tc.tile_pool(name="dram", bufs=1, space="DRAM") as dram,
        ):
            # ---- static SBUF tensors ----
            xT_t = [big.tile([128, BL], BF16, tag=f"xT{k}", name=f"xT{k}")
                    for k in range(KT)]
            wqkv_t = [big.tile([128, 384], BF16, tag=f"wqkv{k}",
                               name=f"wqkv{k}") for k in range(KT)]
            # DMA priority: the 8 wqkv tiles land on queues 0-7 while the
            # first xT b0 column-chunks land on 8-15 in parallel; b0 moves
            # in fine [128, 512] chunks (lowest latency to first scores),
            # b1 in coarser [128, 2048] chunks (fewer descriptors).
            for k in range(KT):
                nc.sync.dma_start(wqkv_t[k][:],
                                  wqkv_ext[k * 128:(k + 1) * 128, :])
            for cc in range(4):
                for k in range(KT):
                    cs = slice(cc * 512, (cc + 1) * 512)
                    nc.sync.dma_start(
                        xT_t[k][:, cs],
                        xT_ext[k * 128:(k + 1) * 128, cs])
            for k in range(KT):
                nc.sync.dma_start(
                    xT_t[k][:, 2048:4096],
                    xT_ext[k * 128:(k + 1) * 128, 2048:4096])

            ident_b = big.tile([128, 128], BF16, tag="ident_b")
            make_identity(nc, ident_b[:])

            # Preload the Exp activation table while DMAs stream in, so the
            # first real exp doesn't pay the ~1.3us table load.  Emitted
            # before the other Pool-engine memsets so it lands early.
            dum = big.tile([1, 16], F32, tag="dum")
            nc.gpsimd.memset(dum[:], 0.0)
            dum2 = big.tile([1, 16], BF16, tag="dum2")
            nc.scalar.activation(dum2[:], dum[:], Exp, scale=1.0)

            # q stacked (h0 rows 0:64, h1 rows 64:128) -- no padding.
            qs_t = big.tile([128, BL], BF16, tag="qs", name="qs")
            # k per head, zero-padded in complementary halves.
            kp_t = [big.tile([128, BL], BF16, tag=f"kp{h}", name=f"kp{h}")
                    for h in range(2)]
            nc.vector.memset(kp_t[0][64:128, :], 0.0)
            nc.vector.memset(kp_t[1][0:64, :], 0.0)
            # v: cols [h*128 : h*128+64] = head h, +64 = ones, rest zero
            v_t = [big.tile([128, 256], BF16, tag=f"v{t}", name=f"v{t}")
                   for t in range(VT)]
            for t in range(VT):
                vv = v_t[t][:].rearrange("p (h c) -> p h c", h=2)
                nc.gpsimd.memset(vv[:, :, 64:65], 1.0)
                nc.gpsimd.memset(vv[:, :, 65:128], 0.0)
            # final o^T for our 2 heads, all 4096 cols
            oT_f = big.tile([128, BL], BF16, tag="oT")
            # received heads for our 512 output rows (cols 0:256 b0, 256:512 b1)
            ogT_t = [big.tile([128, 512], BF16, tag=f"ogT{k}", name=f"ogT{k}")
                     for k in range(KT)]
            # softmax-denominator staging: denom row copied out of PSUM
            # (reciprocal_approx_fast misreads PSUM inputs), its reciprocal,
            # and the partition-broadcast copy; double-buffered by lc parity.
            sdf = [big.tile([1, 512], F32, tag=f"sdf{i}", name=f"sdf{i}")
                   for i in range(2)]
            scrf = [big.tile([1, 512], F32, tag=f"scrf{i}", name=f"scrf{i}")
                    for i in range(2)]
            bcsf = [big.tile([64, 512], F32, tag=f"bcsf{i}", name=f"bcsf{i}")
                    for i in range(2)]

            # Warm the HAM clock gate during the initial DMA wait: ~30
            # back-to-back matmuls lift the PE to 2.4 GHz before the first
            # real matmul issues.  Output is scratch nobody reads.
            warm = pov.tile([128, 512], F32, tag="ov", name="warm")
            for i in range(30):
                nc.tensor.matmul(warm[:, 0:128], ident_b[:], ident_b[:],
                                 start=(i == 0), stop=(i == 29))

            # ---- background PE work generators (woven between scores) ----
            qk_pending = {}  # (m, ncol) -> psum tile with k 0..3 accumulated

            def emit_qk_half(m, ncol, half):
                # m: 0 -> q, 1 -> k.  One 512-col chunk of the projection,
                # split into two 4-matmul half-chains (the PSUM accumulation
                # pauses in between) so background pops stay fine-grained.
                if half == 0:
                    ps = pbc.tile([128, 512], F32, tag="bc",
                                  name=f"qk{m}_{ncol}")
                    qk_pending[(m, ncol)] = ps
                else:
                    ps = qk_pending.pop((m, ncol))
                for k in range(half * 4, half * 4 + 4):
                    nc.tensor.matmul(
                        ps[:],
                        wqkv_t[k][:, m * 128:(m + 1) * 128],
                        xT_t[k][:, ncol * 512:(ncol + 1) * 512],
                        start=(k == 0), stop=(k == KT - 1),
                    )
                if half == 0:
                    return
                cs = slice(ncol * 512, (ncol + 1) * 512)
                if m == 0:
                    nc.vector.tensor_copy(qs_t[:, cs], ps[:])
                else:
                    nc.vector.tensor_copy(kp_t[0][0:64, cs], ps[0:64, :])
                    nc.vector.tensor_copy(kp_t[1][64:128, cs],
                                          ps[64:128, :])

            def emit_qk_col(m, ncol):
                emit_qk_half(m, ncol, 0)
                emit_qk_half(m, ncol, 1)

            def emit_v_tile(t):
                ps = pbc.tile([128, 128], F32, tag="bc", name=f"v_ps{t}")
                for k in range(KT):
                    nc.tensor.matmul(
                        ps[:],
                        xT_t[k][:, t * 128:(t + 1) * 128],
                        wqkv_t[k][:, 256:384],
                        start=(k == 0), stop=(k == KT - 1),
                    )
                vv = v_t[t][:].rearrange("p (h c) -> p h c", h=2)
                nc.vector.tensor_copy(
                    vv[:, :, 0:64],
                    ps[:].rearrange("p (h c) -> p h c", h=2))

            def make_bg():
                # Generator of small PE work chunks, popped between
                # foreground score/attend matmuls.  Ordering constraints:
                # k ncol N before scores (unit 0, lc 0, mp >= 2N); q ncol N
                # before scores of (unit 0, lc N); v tile t of b0 before the
                # attend chunk reading it (block 2, pops precede attend
                # chunks within a slot); qk b1 before unit 2 scores; v b1
                # before unit 2 attends.
                def qk(m, ncol):
                    yield ('qkh', m, ncol, 0)
                    yield ('qkh', m, ncol, 1)

                for ncol in range(1, LC):
                    yield from qk(1, ncol)
                yield from qk(0, 1)
                for t in range(4, VT // 4):
                    yield ('v', t)
                yield from qk(0, 2)
                for t in range(VT // 4, VT // 2):
                    yield ('v', t)
                yield from qk(0, 3)
                for ncol in range(LC, 2 * LC):
                    yield from qk(1, ncol)
                    yield from qk(0, ncol)
                for t in range(VT // 2, VT):
                    yield ('v', t)

            bg = make_bg()
            bg_done = [False]
            bg_deficit = [0.0]  # us of background work we owe the stream
            emitted = {('qk', 1, 0), ('qk', 0, 0)}  # prologue

            def bg_step():
                item = next(bg, None)
                if item is None:
                    bg_done[0] = True
                    return False
                if item[0] == 'v':
                    emitted.add(item)
                    emit_v_tile(item[1])
                    bg_deficit[0] -= 0.55
                else:
                    _, m, ncol, half = item
                    emit_qk_half(m, ncol, half)
                    if half == 1:
                        emitted.add(('qk', m, ncol))
                    bg_deficit[0] -= 0.95
                return True

            def bg_pop(budget_us):
                # emit background chunks worth ~budget_us of PE time
                bg_deficit[0] += budget_us
                while not bg_done[0] and bg_deficit[0] > 0:
                    if not bg_step():
                        return

            def bg_need(item):
                # structural deadline: force-drain the queue until `item`
                # has been emitted (it must precede its first consumer in
                # the in-order PE queue, or the kernel deadlocks).
                while item not in emitted and not bg_done[0]:
                    bg_step()

            # ---- attention unit pieces ----
            # one exchange per batch, fired after that batch's second
            # (hl=1) unit, carrying both head rows: [8, 128, 256]
            cc_in = [dram.tile([N_CORES, 128, 256], BF16, name=f"cc_in{i}")
                     for i in range(2)]
            cc_out = [dram.tile([N_CORES, 128, 256], BF16, name=f"cc_out{i}")
                      for i in range(2)]

            def emit_scores(hl, b, lc, mp):
                # one st tile = S^T for m-tiles (2mp, 2mp+1) x 512 l-cols
                ls = slice(b * L + lc * 512, b * L + (lc + 1) * 512)
                st = pst.tile([128, 1024], F32, tag="st")
                for h2 in range(2):
                    mt = 2 * mp + h2
                    nc.tensor.matmul(
                        st[:, h2 * 512:(h2 + 1) * 512],
                        kp_t[hl][:, b * L + mt * 128:b * L + (mt + 1) * 128],
                        qs_t[:, ls],
                        start=True, stop=True,
                    )
                pt = ptp.tile([128, 2, 512], BF16, tag="pt")
                nc.scalar.activation(pt[:], st[:], Exp, scale=SCALE)
                return pt

            def emit_av_chunk(hl, b, ov, pts, mt0, mt1):
                for mt in range(mt0, mt1):
                    nc.tensor.matmul(
                        ov[:],
                        v_t[b * MT + mt][:, hl * 128:(hl + 1) * 128],
                        pts[mt // 2][:, mt % 2, :],
                        start=(mt == 0), stop=(mt == MT - 1),
                    )

            def emit_norm(u, hl, b, lc, ov):
                # normalize ov rows 0:64 by its colsum row (64) and write
                # bf16 o^T.  The reciprocal row is broadcast across
                # partitions on the idle GpSimd engine, keeping the PE
                # queue out of the chain.  After the second (hl=1) unit of
                # a batch, both head rows of this window are final -- stage
                # the two 256-col a2a slots.
                sd = sdf[lc % 2]
                sf = scrf[lc % 2]
                bf = bcsf[lc % 2]
                nc.vector.tensor_copy(sd[0:1, :], ov[64:65, :])
                nc.vector.reciprocal_approx_fast(sf[0:1, :], sd[0:1, :])
                nc.gpsimd.partition_broadcast(bf[:], sf[0:1, :])
                hs = slice(hl * 64, (hl + 1) * 64)
                win = slice(b * L + lc * 512, b * L + (lc + 1) * 512)
                nc.vector.tensor_mul(oT_f[hs, win], ov[0:64, :], bf[:])
                if hl == 1:
                    for jj in range(2):
                        j = 2 * lc + jj
                        nc.sync.dma_start(
                            cc_in[b][j],
                            oT_f[:, b * L + j * 256:b * L + (j + 1) * 256])

            def emit_exchange(b):
                nc.gpsimd.collective_compute(
                    "AllToAll",
                    mybir.AluOpType.bypass,
                    ins=[cc_in[b].opt()],
                    outs=[cc_out[b].opt()],
                    replica_groups=[list(range(N_CORES))],
                )
                for k in range(N_CORES):
                    for h in range(2):
                        nc.sync.dma_start(
                            ogT_t[k][h * 64:(h + 1) * 64,
                                     b * 256:(b + 1) * 256],
                            cc_out[b][k][h * 64:(h + 1) * 64, :])

            # ---- emission schedule ----
            # Prologue: only k and q for ncol 0 -- the first scores matmul
            # needs just those, so the exp stream starts ~10us in; the
            # remaining k/q columns, all v tiles and batch-1 qkv are woven
            # between foreground matmuls via the background queue, each
            # ahead of its first consumer.
            emit_qk_col(1, 0)
            emit_qk_col(0, 0)
            for t in range(4):  # v tiles of cc0 -- same DMA dep, PE idle
                emitted.add(('v', t))
                emit_v_tile(t)

            # Blocks: one per (unit, lc).  Block i emits scores for chunk i
            # and (woven between them) the attend chain for chunk i-2 --
            # the lag keeps the PE queue from deadlocking on pt buffers and
            # gives the background queue room in the first two blocks.
            chunks = [(u, hl, b, lc)
                      for u, (hl, b) in enumerate(UNITS) for lc in range(LC)]
            pts_of = {}
            n_chunks = len(chunks)
            AVLAG = 2

            for i in range(n_chunks + AVLAG):
                sc = chunks[i] if i < n_chunks else None
                avi = i - AVLAG
                av = chunks[avi] if avi >= 0 else None
                if av is not None:
                    au, ahl, ab, alc = av
                    aov = pov.tile([128, 512], F32, tag="ov")
                    apts = pts_of.pop(avi)
                if sc is not None:
                    su, shl, sb, slc = sc
                    bg_need(('qk', 0, sb * LC + slc))  # q cols of this lc
                    pts = []
                    for mp in range(MT // 2):
                        bg_need(('qk', 1, sb * LC + mp // 2))  # k m-tiles
                        pts.append(emit_scores(shl, sb, slc, mp))
                        if av is not None:
                            bg_pop(0.5)
                            bg_need(('v', ab * MT + 2 * mp + 1))
                            emit_av_chunk(ahl, ab, aov, apts,
                                          2 * mp, 2 * mp + 2)
                        else:
                            bg_pop(1.0)
                    pts_of[i] = pts
                else:
                    # tail blocks: attend only (exp stream is draining)
                    bg_need(('v', ab * MT + MT - 1))
                    emit_av_chunk(ahl, ab, aov, apts, 0, MT)
                if av is not None:
                    emit_norm(au, ahl, ab, alc, aov)
                    if alc == LC - 1 and ahl == 1:
                        emit_exchange(ab)
            bg_pop(100)  # safety: drain any background leftovers

            # ---- output projection for our 512 rows ----
            # lt 0,1 = b0 (gated on exchanges 0,1 -- long arrived, runs
            # during the final exchange); lt 2,3 = b1 (gated on exchange 3).
            wout_t = [big.tile([128, D], BF16, tag=f"xT{k}", name=f"wout{k}")
                      for k in range(KT)]
            for k in range(KT):
                nc.sync.dma_start(wout_t[k][:],
                                  wout_ext[k * 128:(k + 1) * 128, :])
            for lt in range(4):
                for nt in range(2):
                    ps = pst.tile([128, 1024], F32, tag="st")
                    for k in range(KT):
                        nc.tensor.matmul(
                            ps[:, 0:512],
                            ogT_t[k][:, lt * 128:(lt + 1) * 128],
                            wout_t[k][:, nt * 512:(nt + 1) * 512],
                            start=(k == 0), stop=(k == KT - 1),
                        )
                    osb = small.tile([128, 512], BF16, tag="osb")
                    # alternate copy engines: ScalarE is idle at the tail
                    if nt == 0:
                        nc.vector.tensor_copy(osb[:], ps[:, 0:512])
                    else:
                        nc.scalar.copy(osb[:], ps[:, 0:512])
                    for h in range(2):
                        nc.sync.dma_start(
                            out_ext[lt * 128 + h * 64:lt * 128 + (h + 1) * 64,
                                    nt * 512:(nt + 1) * 512],
                            osb[h * 64:(h + 1) * 64, :])

    nc.compile()
    return nc


_NC_CACHE = None


def _get_nc():
    global _NC_CACHE
    if _NC_CACHE is None:
        _NC_CACHE = _build()
    return _NC_CACHE


def _make_in_maps(x, w_qkv, w_out):
    x = np.asarray(x, dtype=np.float32)
    w_qkv = np.asarray(w_qkv, dtype=np.float32)
    w_out = np.asarray(w_out, dtype=np.float32)
    bf = ml_dtypes.bfloat16
    xT = np.ascontiguousarray(
        x.transpose(2, 0, 1).reshape(D, BL)).astype(bf)
    wout_b = w_out.astype(bf)
    in_maps = []
    for c in range(N_CORES):
        cs = slice(c * 128, (c + 1) * 128)
        wqkv_c = np.ascontiguousarray(
            np.concatenate([w_qkv[:, cs], w_qkv[:, D:][:, cs],
                            w_qkv[:, 2 * D:][:, cs]], axis=1)
        ).astype(bf)
        in_maps.append({"xT": xT, "wqkv": wqkv_c, "wout": wout_b})
    return in_maps


def _run(x, w_qkv, w_out, trace=False):
    nc = _get_nc()
    in_maps = _make_in_maps(x, w_qkv, w_out)
    res = bass_utils.run_bass_kernel_spmd(
        nc, in_maps, list(range(N_CORES)), trace=trace)
    out = np.empty((B, L, D), dtype=np.float32)
    for c in range(N_CORES):
        r = np.asarray(res.results[c]["out"]).astype(np.float32)
        out[0, c * 256:(c + 1) * 256, :] = r[0:256]
        out[1, c * 256:(c + 1) * 256, :] = r[256:512]
    return out, res


def kernel(x, w_qkv, w_out):
    out, _ = _run(x, w_qkv, w_out, trace=False)
    return out


# revision 55
# speedup vs baseline: 1.0351x; 1.0302x over previous
"""Multi-head attention on 8 TRN2 NeuronCores.

Problem: x[2, 2048, 1024], w_qkv[1024, 3072], w_out[1024, 1024] (f32).
  qkv = x @ w_qkv; q,k,v per 16 heads of dim 64; softmax(q k^T / 8) v; out proj.

Sharding: 16 heads split 8 ways (one head-PAIR per core, both batches on
every core).  Output ownership is interleaved by batch: core c owns rows
(b=0, cols c*256:(c+1)*256) and (b=1, same cols), so that after each
(head, batch) attention unit a small AllToAll of [8, 64, 256] can fire in
which EVERY slot carries final data.  Four such exchanges; only the last
is exposed, and it overlaps the b=0 half of the output projection.

The kernel is scheduled around the Scalar engine (exp is 140us of the
~190us total work): the k-projection for batch 0 is emitted first so the
first scores matmul (and exp) issues at ~14us, and every other piece of
PE work (q cols, v tiles, batch-1 qkv, broadcast matmuls) is woven
between score/attend matmuls so the exp stream never stalls on a block
of projection work.

Layout: scores are computed TRANSPOSED (S^T[m, l] tiles); softmax sum
over the partition axis comes free from a ones-column appended to v in
the attn@v matmul (out rows = [o^T; colsums]).  exp() runs on ScalarE
straight out of PSUM with the 1/8 scale folded in.  Normalization:
recip(colsums) -> bf16 row, a ones-row matmul broadcasts it across 64
partitions, one DVE multiply writes normalized o^T bf16 -- no transposes.

q is stored STACKED ([q_h0; q_h1] on 128 partitions, no zero pad); the
per-head k tiles are zero-padded in complementary halves (k_h0 on rows
0:64, k_h1 on rows 64:128) so each scores matmul contracts over the full
128 partitions (keeps the HAM clock gate at 2.4 GHz) while the other
head's q rows are killed by zero weights.

Compute dtype bf16 (f32 accumulation in PSUM).
"""

import sys
import types

sys.path.insert(0, "/opt/trn_rl_repo")

import numpy as np
import ml_dtypes

import concourse.bass as bass
import concourse.mybir as mybir
import concourse.tile as tile
from concourse import bacc
from concourse import bass_utils
from concourse.masks import make_identity

# If the image's antenv lacks the axon_hooks module, run_bass_kernel_spmd's
# trace path (reachable via BASS_TRACE=1) would die on import.  Provide the
# registry so tracing degrades gracefully instead (hook stays None unless
# trn_boot registered one).
try:
    import antenv.axon_hooks  # noqa: F401
except ImportError:
    _hooks = types.ModuleType("antenv.axon_hooks")
    _hooks._hook = None
    _hooks.set_axon_ntff_profile_hook = (
        lambda h: setattr(_hooks, "_hook", h))
    _hooks.get_axon_ntff_profile_hook = lambda: _hooks._hook
    sys.modules["antenv.axon_hooks"] = _hooks

# Artifact upload needs bucket credentials; fall back to the local dir so a
# traced run in a sandboxed container still completes.
_orig_upload = bass_utils.upload_artifacts


def _safe_upload(tmpdir):
    try:
        return _orig_upload(tmpdir)
    except Exception:
        return tmpdir


bass_utils.upload_artifacts = _safe_upload

B, L, D, H, DH = 2, 2048, 1024, 16, 64
BL = B * L  # 4096
SCALE = DH ** -0.5
N_CORES = 8
BF16 = mybir.dt.bfloat16
F32 = mybir.dt.float32
Exp = mybir.ActivationFunctionType.Exp

KT = D // 128          # 8 k-tiles over the model dim
MT = L // 128          # 16 m-tiles per batch
LC = L // 512          # 4 l-chunks of 512 per batch
VT = BL // 128         # 32 v row-tiles over (b, l)
# units in emission order: (hl, b)
UNITS = [(0, 0), (1, 0), (0, 1), (1, 1)]


def _build():
    nc = bacc.Bacc("TRN2", target_bir_lowering=False, debug=False,
                   num_devices=N_CORES)
    xT_ext = nc.declare_dram_parameter("xT", [D, BL], BF16, isOutput=False)
    # q (128) | k (128) | v (128) columns for this core's head pair
    wqkv_ext = nc.declare_dram_parameter("wqkv", [D, 384], BF16,
                                         isOutput=False)
    wout_ext = nc.declare_dram_parameter("wout", [D, D], BF16, isOutput=False)
    out_ext = nc.declare_dram_parameter("out", [512, D], BF16, isOutput=True)

    with tile.TileContext(nc) as tc:
        with (
            tc.tile_pool(name="big", bufs=1) as big,
            tc.tile_pool(name="pt", bufs=24) as ptp,
            tc.tile_pool(name="small", bufs=3) as small,
            tc.tile_pool(name="psum_st", bufs=3, space="PSUM") as pst,
            tc.tile_pool(name="psum_ov", bufs=2, space="PSUM") as pov,
            tc.tile_pool(name="dram", bufs=1, space="DRAM") as dram,
        ):
            # ---- static SBUF tensors ----
            xT_t = [big.tile([128, BL], BF16, tag=f"xT{k}", name=f"xT{k}")
                    for k in range(KT)]
            wqkv_t = [big.tile([128, 384], BF16, tag=f"wqkv{k}",
                               name=f"wqkv{k}") for k in range(KT)]
            # DMA priority: the 8 wqkv tiles land on queues 0-7 while the
            # first xT b0 column-chunks land on 8-15 in parallel; b0 moves
            # in fine [128, 512] chunks (lowest latency to first scores),
            # b1 in coarser [128, 2048] chunks (fewer descriptors).
            for k in range(KT):
                nc.sync.dma_start(wqkv_t[k][:],
                                  wqkv_ext[k * 128:(k + 1) * 128, :])
            for cc in range(4):
                for k in range(KT):
                    cs = slice(cc * 512, (cc + 1) * 512)
                    nc.sync.dma_start(
                        xT_t[k][:, cs],
                        xT_ext[k * 128:(k + 1) * 128, cs])
            for k in range(KT):
                nc.sync.dma_start(
                    xT_t[k][:, 2048:4096],
                    xT_ext[k * 128:(k + 1) * 128, 2048:4096])

            ident_b = big.tile([128, 128], BF16, tag="ident_b")
            make_identity(nc, ident_b[:])

            # Preload the Exp activation table while DMAs stream in, so the
            # first real exp doesn't pay the ~1.3us table load.  Emitted
            # before the other Pool-engine memsets so it lands early.
            dum = big.tile([1, 16], F32, tag="dum")
            nc.gpsimd.memset(dum[:], 0.0)
            dum2 = big.tile([1, 16], BF16, tag="dum2")
            nc.scalar.activation(dum2[:], dum[:], Exp, scale=1.0)

            # q stacked (h0 rows 0:64, h1 rows 64:128) -- no padding.
            qs_t = big.tile([128, BL], BF16, tag="qs", name="qs")
            # k per head, zero-padded in complementary halves.
            kp_t = [big.tile([128, BL], BF16, tag=f"kp{h}", name=f"kp{h}")
                    for h in range(2)]
            nc.vector.memset(kp_t[0][64:128, :], 0.0)
            nc.vector.memset(kp_t[1][0:64, :], 0.0)
            # v: cols [h*128 : h*128+64] = head h, +64 = ones, rest zero
            v_t = [big.tile([128, 256], BF16, tag=f"v{t}", name=f"v{t}")
                   for t in range(VT)]
            for t in range(VT):
                vv = v_t[t][:].rearrange("p (h c) -> p h c", h=2)
                nc.gpsimd.memset(vv[:, :, 64:65], 1.0)
                nc.gpsimd.memset(vv[:, :, 65:128], 0.0)
            # final o^T for our 2 heads, all 4096 cols
            oT_f = big.tile([128, BL], BF16, tag="oT")
            # received heads for our 512 output rows (cols 0:256 b0, 256:512 b1)
            ogT_t = [big.tile([128, 512], BF16, tag=f"ogT{k}", name=f"ogT{k}")
                     for k in range(KT)]
            # softmax-denominator staging: denom row copied out of PSUM
            # (reciprocal_approx_fast misreads PSUM inputs), its reciprocal,
            # and the partition-broadcast copy; double-buffered by lc parity.
            sdf = [big.tile([1, 512], F32, tag=f"sdf{i}", name=f"sdf{i}")
                   for i in range(2)]
            scrf = [big.tile([1, 512], F32, tag=f"scrf{i}", name=f"scrf{i}")
                    for i in range(2)]
            bcsf = [big.tile([64, 512], F32, tag=f"bcsf{i}", name=f"bcsf{i}")
                    for i in range(2)]

            # Warm the HAM clock gate during the initial DMA wait: ~30
            # back-to-back matmuls lift the PE to 2.4 GHz before the first
            # real matmul issues.  Output is scratch nobody reads.
            warm = pov.tile([128, 512], F32, tag="ov", name="warm")
            for i in range(30):
                nc.tensor.matmul(warm[:, 0:128], ident_b[:], ident_b[:],
                                 start=(i == 0), stop=(i == 29))

            # ---- background PE work generators (woven between scores) ----
            def emit_qk_col(m, ncol):
                # m: 0 -> q, 1 -> k.  One 512-col chunk of the projection.
                ps = pst.tile([128, 512], F32, tag="st",
                              name=f"qk{m}_{ncol}")
                for k in range(KT):
                    nc.tensor.matmul(
                        ps[:],
                        wqkv_t[k][:, m * 128:(m + 1) * 128],
                        xT_t[k][:, ncol * 512:(ncol + 1) * 512],
                        start=(k == 0), stop=(k == KT - 1),
                    )
                cs = slice(ncol * 512, (ncol + 1) * 512)
                if m == 0:
                    nc.vector.tensor_copy(qs_t[:, cs], ps[:])
                else:
                    nc.vector.tensor_copy(kp_t[0][0:64, cs], ps[0:64, :])
                    nc.vector.tensor_copy(kp_t[1][64:128, cs],
                                          ps[64:128, :])

            def emit_v_tile(t):
                ps = pst.tile([128, 128], F32, tag="st", name=f"v_ps{t}")
                for k in range(KT):
                    nc.tensor.matmul(
                        ps[:],
                        xT_t[k][:, t * 128:(t + 1) * 128],
                        wqkv_t[k][:, 256:384],
                        start=(k == 0), stop=(k == KT - 1),
                    )
                vv = v_t[t][:].rearrange("p (h c) -> p h c", h=2)
                nc.vector.tensor_copy(
                    vv[:, :, 0:64],
                    ps[:].rearrange("p (h c) -> p h c", h=2))

            def make_bg():
                # Generator of small PE work chunks, popped between
                # foreground score/attend matmuls.  Ordering constraints:
                # k ncol N before scores (unit 0, lc 0, mp >= 2N); q ncol N
                # before scores of (unit 0, lc N); v tile t of b0 before the
                # attend chunk reading it (block 2, pops precede attend
                # chunks within a slot); qk b1 before unit 2 scores; v b1
                # before unit 2 attends.
                for ncol in range(1, LC):
                    yield ('qk', 1, ncol)
                yield ('qk', 0, 1)
                for t in range(4, VT // 4):
                    yield ('v', t)
                yield ('qk', 0, 2)
                for t in range(VT // 4, VT // 2):
                    yield ('v', t)
                yield ('qk', 0, 3)
                for ncol in range(LC, 2 * LC):
                    yield ('qk', 1, ncol)
                    yield ('qk', 0, ncol)
                for t in range(VT // 2, VT):
                    yield ('v', t)

            bg = make_bg()
            bg_done = [False]
            bg_deficit = [0.0]  # us of background work we owe the stream
            emitted = {('qk', 1, 0), ('qk', 0, 0)}  # prologue

            def bg_step():
                item = next(bg, None)
                if item is None:
                    bg_done[0] = True
                    return False
                emitted.add(item)
                if item[0] == 'v':
                    emit_v_tile(item[1])
                    bg_deficit[0] -= 0.55
                else:
                    emit_qk_col(item[1], item[2])
                    bg_deficit[0] -= 1.9
                return True

            def bg_pop(budget_us):
                # emit background chunks worth ~budget_us of PE time
                bg_deficit[0] += budget_us
                while not bg_done[0] and bg_deficit[0] > 0:
                    if not bg_step():
                        return

            def bg_need(item):
                # structural deadline: force-drain the queue until `item`
                # has been emitted (it must precede its first consumer in
                # the in-order PE queue, or the kernel deadlocks).
                while item not in emitted and not bg_done[0]:
                    bg_step()

            # ---- attention unit pieces ----
            # one exchange per batch, fired after that batch's second
            # (hl=1) unit, carrying both head rows: [8, 128, 256]
            cc_in = [dram.tile([N_CORES, 128, 256], BF16, name=f"cc_in{i}")
                     for i in range(2)]
            cc_out = [dram.tile([N_CORES, 128, 256], BF16, name=f"cc_out{i}")
                      for i in range(2)]

            def emit_scores(hl, b, lc, mp):
                # one st tile = S^T for m-tiles (2mp, 2mp+1) x 512 l-cols
                ls = slice(b * L + lc * 512, b * L + (lc + 1) * 512)
                st = pst.tile([128, 1024], F32, tag="st")
                for h2 in range(2):
                    mt = 2 * mp + h2
                    nc.tensor.matmul(
                        st[:, h2 * 512:(h2 + 1) * 512],
                        kp_t[hl][:, b * L + mt * 128:b * L + (mt + 1) * 128],
                        qs_t[:, ls],
                        start=True, stop=True,
                    )
                pt = ptp.tile([128, 2, 512], BF16, tag="pt")
                nc.scalar.activation(pt[:], st[:], Exp, scale=SCALE)
                return pt

            def emit_av_chunk(hl, b, ov, pts, mt0, mt1):
                for mt in range(mt0, mt1):
                    nc.tensor.matmul(
                        ov[:],
                        v_t[b * MT + mt][:, hl * 128:(hl + 1) * 128],
                        pts[mt // 2][:, mt % 2, :],
                        start=(mt == 0), stop=(mt == MT - 1),
                    )

            def emit_norm(u, hl, b, lc, ov):
                # normalize ov rows 0:64 by its colsum row (64) and write
                # bf16 o^T.  The reciprocal row is broadcast across
                # partitions on the idle GpSimd engine, keeping the PE
                # queue out of the chain.  After the second (hl=1) unit of
                # a batch, both head rows of this window are final -- stage
                # the two 256-col a2a slots.
                sd = sdf[lc % 2]
                sf = scrf[lc % 2]
                bf = bcsf[lc % 2]
                nc.vector.tensor_copy(sd[0:1, :], ov[64:65, :])
                nc.vector.reciprocal_approx_fast(sf[0:1, :], sd[0:1, :])
                nc.gpsimd.partition_broadcast(bf[:], sf[0:1, :])
                hs = slice(hl * 64, (hl + 1) * 64)
                win = slice(b * L + lc * 512, b * L + (lc + 1) * 512)
                nc.vector.tensor_mul(oT_f[hs, win], ov[0:64, :], bf[:])
                if hl == 1:
                    for jj in range(2):
                        j = 2 * lc + jj
                        nc.sync.dma_start(
                            cc_in[b][j],
                            oT_f[:, b * L + j * 256:b * L + (j + 1) * 256])

            def emit_exchange(b):
                nc.gpsimd.collective_compute(
                    "AllToAll",
                    mybir.AluOpType.bypass,
                    ins=[cc_in[b].opt()],
                    outs=[cc_out[b].opt()],
                    replica_groups=[list(range(N_CORES))],
                )
                for k in range(N_CORES):
                    for h in range(2):
                        nc.sync.dma_start(
                            ogT_t[k][h * 64:(h + 1) * 64,
                                     b * 256:(b + 1) * 256],
                            cc_out[b][k][h * 64:(h + 1) * 64, :])

            # ---- emission schedule ----
            # Prologue: only k and q for ncol 0 -- the first scores matmul
            # needs just those, so the exp stream starts ~10us in; the
            # remaining k/q columns, all v tiles and batch-1 qkv are woven
            # between foreground matmuls via the background queue, each
            # ahead of its first consumer.
            emit_qk_col(1, 0)
            emit_qk_col(0, 0)
            for t in range(4):  # v tiles of cc0 -- same DMA dep, PE idle
                emitted.add(('v', t))
                emit_v_tile(t)

            # Blocks: one per (unit, lc).  Block i emits scores for chunk i
            # and (woven between them) the attend chain for chunk i-2 --
            # the lag keeps the PE queue from deadlocking on pt buffers and
            # gives the background queue room in the first two blocks.
            chunks = [(u, hl, b, lc)
                      for u, (hl, b) in enumerate(UNITS) for lc in range(LC)]
            pts_of = {}
            n_chunks = len(chunks)
            AVLAG = 2

            for i in range(n_chunks + AVLAG):
                sc = chunks[i] if i < n_chunks else None
                avi = i - AVLAG
                av = chunks[avi] if avi >= 0 else None
                if av is not None:
                    au, ahl, ab, alc = av
                    aov = pov.tile([128, 512], F32, tag="ov")
                    apts = pts_of.pop(avi)
                if sc is not None:
                    su, shl, sb, slc = sc
                    bg_need(('qk', 0, sb * LC + slc))  # q cols of this lc
                    pts = []
                    for mp in range(MT // 2):
                        bg_need(('qk', 1, sb * LC + mp // 2))  # k m-tiles
                        pts.append(emit_scores(shl, sb, slc, mp))
                        if av is not None:
                            bg_pop(0.5)
                            bg_need(('v', ab * MT + 2 * mp + 1))
                            emit_av_chunk(ahl, ab, aov, apts,
                                          2 * mp, 2 * mp + 2)
                        else:
                            bg_pop(1.0)
                    pts_of[i] = pts
                else:
                    # tail blocks: attend only (exp stream is draining)
                    bg_need(('v', ab * MT + MT - 1))
                    emit_av_chunk(ahl, ab, aov, apts, 0, MT)
                if av is not None:
                    emit_norm(au, ahl, ab, alc, aov)
                    if alc == LC - 1 and ahl == 1:
                        emit_exchange(ab)
            bg_pop(100)  # safety: drain any background leftovers

            # ---- output projection for our 512 rows ----
            # lt 0,1 = b0 (gated on exchanges 0,1 -- long arrived, runs
            # during the final exchange); lt 2,3 = b1 (gated on exchange 3).
            wout_t = [big.tile([128, D], BF16, tag=f"xT{k}", name=f"wout{k}")
                      for k in range(KT)]
            for k in range(KT):
                nc.sync.dma_start(wout_t[k][:],
                                  wout_ext[k * 128:(k + 1) * 128, :])
            for lt in range(4):
                for nt in range(2):
                    ps = pst.tile([128, 1024], F32, tag="st")
                    for k in range(KT):
                        nc.tensor.matmul(
                            ps[:, 0:512],
                            ogT_t[k][:, lt * 128:(lt + 1) * 128],
                            wout_t[k][:, nt * 512:(nt + 1) * 512],
                            start=(k == 0), stop=(k == KT - 1),
                        )
                    osb = small.tile([128, 512], BF16, tag="osb")
                    # alternate copy engines: ScalarE is idle at the tail
                    if nt == 0:
                        nc.vector.tensor_copy(osb[:], ps[:, 0:512])
                    else:
                        nc.scalar.copy(osb[:], ps[:, 0:512])
                    for h in range(2):
                        nc.sync.dma_start(
                            out_ext[lt * 128 + h * 64:lt * 128 + (h + 1) * 64,
                                    nt * 512:(nt + 1) * 512],
                            osb[h * 64:(h + 1) * 64, :])

    nc.compile()
    return nc


_NC_CACHE = None


def _get_nc():
    global _NC_CACHE
    if _NC_CACHE is None:
        _NC_CACHE = _build()
    return _NC_CACHE


def _make_in_maps(x, w_qkv, w_out):
    x = np.asarray(x, dtype=np.float32)
    w_qkv = np.asarray(w_qkv, dtype=np.float32)
    w_out = np.asarray(w_out, dtype=np.float32)
    bf = ml_dtypes.bfloat16
    xT = np.ascontiguousarray(
        x.transpose(2, 0, 1).reshape(D, BL)).astype(bf)
    wout_b = w_out.astype(bf)
    in_maps = []
    for c in range(N_CORES):
        cs = slice(c * 128, (c + 1) * 128)
        wqkv_c = np.ascontiguousarray(
            np.concatenate([w_qkv[:, cs], w_qkv[:, D:][:, cs],
                            w_qkv[:, 2 * D:][:, cs]], axis=1)
        ).astype(bf)
        in_maps.append({"xT": xT, "wqkv": wqkv_c, "wout": wout_b})
    return in_maps


def _run(x, w_qkv, w_out, trace=False):
    nc = _get_nc()
    in_maps = _make_in_maps(x, w_qkv, w_out)
    res = bass_utils.run_bass_kernel_spmd(
        nc, in_maps, list(range(N_CORES)), trace=trace)
    out = np.empty((B, L, D), dtype=np.float32)
    for c in range(N_CORES):
        r = np.asarray(res.results[c]["out"]).astype(np.float32)
        out[0, c * 256:(c + 1) * 256, :] = r[0:256]
        out[1, c * 256:(c + 1) * 256, :] = r[256:512]
    return out, res


def kernel(x, w_qkv, w_out):
    out, _ = _run(x, w_qkv, w_out, trace=False)
    return out


# revision 56
# speedup vs baseline: 1.0447x; 1.0094x over previous
"""Multi-head attention on 8 TRN2 NeuronCores.

Problem: x[2, 2048, 1024], w_qkv[1024, 3072], w_out[1024, 1024] (f32).
  qkv = x @ w_qkv; q,k,v per 16 heads of dim 64; softmax(q k^T / 8) v; out proj.

Sharding: 16 heads split 8 ways (one head-PAIR per core, both batches on
every core).  Output ownership is interleaved by batch: core c owns rows
(b=0, cols c*256:(c+1)*256) and (b=1, same cols), so that after each
(head, batch) attention unit a small AllToAll of [8, 64, 256] can fire in
which EVERY slot carries final data.  Four such exchanges; only the last
is exposed, and it overlaps the b=0 half of the output projection.

The kernel is scheduled around the Scalar engine (exp is 140us of the
~190us total work): the k-projection for batch 0 is emitted first so the
first scores matmul (and exp) issues at ~14us, and every other piece of
PE work (q cols, v tiles, batch-1 qkv, broadcast matmuls) is woven
between score/attend matmuls so the exp stream never stalls on a block
of projection work.

Layout: scores are computed TRANSPOSED (S^T[m, l] tiles); softmax sum
over the partition axis comes free from a ones-column appended to v in
the attn@v matmul (out rows = [o^T; colsums]).  exp() runs on ScalarE
straight out of PSUM with the 1/8 scale folded in.  Normalization:
recip(colsums) -> bf16 row, a ones-row matmul broadcasts it across 64
partitions, one DVE multiply writes normalized o^T bf16 -- no transposes.

q is stored STACKED ([q_h0; q_h1] on 128 partitions, no zero pad); the
per-head k tiles are zero-padded in complementary halves (k_h0 on rows
0:64, k_h1 on rows 64:128) so each scores matmul contracts over the full
128 partitions (keeps the HAM clock gate at 2.4 GHz) while the other
head's q rows are killed by zero weights.

Compute dtype bf16 (f32 accumulation in PSUM).
"""

import sys
import types

sys.path.insert(0, "/opt/trn_rl_repo")

import numpy as np
import ml_dtypes

import concourse.bass as bass
import concourse.mybir as mybir
import concourse.tile as tile
from concourse import bacc
from concourse import bass_utils
from concourse.masks import make_identity

# If the image's antenv lacks the axon_hooks module, run_bass_kernel_spmd's
# trace path (reachable via BASS_TRACE=1) would die on import.  Provide the
# registry so tracing degrades gracefully instead (hook stays None unless
# trn_boot registered one).
try:
    import antenv.axon_hooks  # noqa: F401
except ImportError:
    _hooks = types.ModuleType("antenv.axon_hooks")
    _hooks._hook = None
    _hooks.set_axon_ntff_profile_hook = (
        lambda h: setattr(_hooks, "_hook", h))
    _hooks.get_axon_ntff_profile_hook = lambda: _hooks._hook
    sys.modules["antenv.axon_hooks"] = _hooks

# Artifact upload needs bucket credentials; fall back to the local dir so a
# traced run in a sandboxed container still completes.
_orig_upload = bass_utils.upload_artifacts


def _safe_upload(tmpdir):
    try:
        return _orig_upload(tmpdir)
    except Exception:
        return tmpdir


bass_utils.upload_artifacts = _safe_upload

B, L, D, H, DH = 2, 2048, 1024, 16, 64
BL = B * L  # 4096
SCALE = DH ** -0.5
N_CORES = 8
BF16 = mybir.dt.bfloat16
F32 = mybir.dt.float32
Exp = mybir.ActivationFunctionType.Exp

KT = D // 128          # 8 k-tiles over the model dim
MT = L // 128          # 16 m-tiles per batch
LC = L // 512          # 4 l-chunks of 512 per batch
VT = BL // 128         # 32 v row-tiles over (b, l)
# units in emission order: (hl, b)
UNITS = [(0, 0), (1, 0), (0, 1), (1, 1)]


def _build():
    nc = bacc.Bacc("TRN2", target_bir_lowering=False, debug=False,
                   num_devices=N_CORES)
    xT_ext = nc.declare_dram_parameter("xT", [D, BL], BF16, isOutput=False)
    # q (128) | k (128) | v (128) columns for this core's head pair
    wqkv_ext = nc.declare_dram_parameter("wqkv", [D, 384], BF16,
                                         isOutput=False)
    wout_ext = nc.declare_dram_parameter("wout", [D, D], BF16, isOutput=False)
    out_ext = nc.declare_dram_parameter("out", [512, D], BF16, isOutput=True)

    with tile.TileContext(nc) as tc:
        with (
            tc.tile_pool(name="big", bufs=1) as big,
            tc.tile_pool(name="pt", bufs=24) as ptp,
            tc.tile_pool(name="small", bufs=3) as small,
            tc.tile_pool(name="psum_st", bufs=3, space="PSUM") as pst,
            tc.tile_pool(name="psum_ov", bufs=2, space="PSUM") as pov,
            tc.tile_pool(name="dram", bufs=1, space="DRAM") as dram,
        ):
            # ---- static SBUF tensors ----
            xT_t = [big.tile([128, BL], BF16, tag=f"xT{k}", name=f"xT{k}")
                    for k in range(KT)]
            wqkv_t = [big.tile([128, 384], BF16, tag=f"wqkv{k}",
                               name=f"wqkv{k}") for k in range(KT)]
            # DMA priority: the 8 wqkv tiles land on queues 0-7 while the
            # first xT b0 column-chunks land on 8-15 in parallel; b0 moves
            # in fine [128, 512] chunks (lowest latency to first scores),
            # b1 in coarser [128, 2048] chunks (fewer descriptors).
            for k in range(KT):
                nc.sync.dma_start(wqkv_t[k][:],
                                  wqkv_ext[k * 128:(k + 1) * 128, :])
            for cc in range(4):
                for k in range(KT):
                    cs = slice(cc * 512, (cc + 1) * 512)
                    nc.sync.dma_start(
                        xT_t[k][:, cs],
                        xT_ext[k * 128:(k + 1) * 128, cs])
            for k in range(KT):
                nc.sync.dma_start(
                    xT_t[k][:, 2048:4096],
                    xT_ext[k * 128:(k + 1) * 128, 2048:4096])

            ident_b = big.tile([128, 128], BF16, tag="ident_b")
            make_identity(nc, ident_b[:])

            # Preload the Exp activation table while DMAs stream in, so the
            # first real exp doesn't pay the ~1.3us table load.  Emitted
            # before the other Pool-engine memsets so it lands early.
            dum = big.tile([1, 16], F32, tag="dum")
            nc.gpsimd.memset(dum[:], 0.0)
            dum2 = big.tile([1, 16], BF16, tag="dum2")
            nc.scalar.activation(dum2[:], dum[:], Exp, scale=1.0)

            # q stacked (h0 rows 0:64, h1 rows 64:128) -- no padding.
            qs_t = big.tile([128, BL], BF16, tag="qs", name="qs")
            # k per head, zero-padded in complementary halves.
            kp_t = [big.tile([128, BL], BF16, tag=f"kp{h}", name=f"kp{h}")
                    for h in range(2)]
            nc.vector.memset(kp_t[0][64:128, :], 0.0)
            nc.vector.memset(kp_t[1][0:64, :], 0.0)
            # v: cols [h*128 : h*128+64] = head h, +64 = ones, rest zero
            v_t = [big.tile([128, 256], BF16, tag=f"v{t}", name=f"v{t}")
                   for t in range(VT)]
            for t in range(VT):
                vv = v_t[t][:].rearrange("p (h c) -> p h c", h=2)
                nc.gpsimd.memset(vv[:, :, 64:65], 1.0)
                nc.gpsimd.memset(vv[:, :, 65:128], 0.0)
            # final o^T for our 2 heads, all 4096 cols
            oT_f = big.tile([128, BL], BF16, tag="oT")
            # received heads for our 512 output rows (cols 0:256 b0, 256:512 b1)
            ogT_t = [big.tile([128, 512], BF16, tag=f"ogT{k}", name=f"ogT{k}")
                     for k in range(KT)]
            # softmax-denominator staging: denom row copied out of PSUM
            # (reciprocal_approx_fast misreads PSUM inputs), its reciprocal,
            # and the partition-broadcast copy; double-buffered by lc parity.
            sdf = [big.tile([1, 512], F32, tag=f"sdf{i}", name=f"sdf{i}")
                   for i in range(2)]
            scrf = [big.tile([1, 512], F32, tag=f"scrf{i}", name=f"scrf{i}")
                    for i in range(2)]
            bcsf = [big.tile([64, 512], F32, tag=f"bcsf{i}", name=f"bcsf{i}")
                    for i in range(2)]

            # Warm the HAM clock gate during the initial DMA wait: ~30
            # back-to-back matmuls lift the PE to 2.4 GHz before the first
            # real matmul issues.  Output is scratch nobody reads.
            warm = pov.tile([128, 512], F32, tag="ov", name="warm")
            for i in range(30):
                nc.tensor.matmul(warm[:, 0:128], ident_b[:], ident_b[:],
                                 start=(i == 0), stop=(i == 29))

            # ---- background PE work generators (woven between scores) ----
            def emit_qk_col(m, ncol):
                # m: 0 -> q, 1 -> k.  One 512-col chunk of the projection.
                ps = pst.tile([128, 512], F32, tag="st",
                              name=f"qk{m}_{ncol}")
                for k in range(KT):
                    nc.tensor.matmul(
                        ps[:],
                        wqkv_t[k][:, m * 128:(m + 1) * 128],
                        xT_t[k][:, ncol * 512:(ncol + 1) * 512],
                        start=(k == 0), stop=(k == KT - 1),
                    )
                cs = slice(ncol * 512, (ncol + 1) * 512)
                if m == 0:
                    nc.vector.tensor_copy(qs_t[:, cs], ps[:])
                else:
                    nc.vector.tensor_copy(kp_t[0][0:64, cs], ps[0:64, :])
                    nc.vector.tensor_copy(kp_t[1][64:128, cs],
                                          ps[64:128, :])

            def emit_v_tile(t):
                ps = pst.tile([128, 128], F32, tag="st", name=f"v_ps{t}")
                for k in range(KT):
                    nc.tensor.matmul(
                        ps[:],
                        xT_t[k][:, t * 128:(t + 1) * 128],
                        wqkv_t[k][:, 256:384],
                        start=(k == 0), stop=(k == KT - 1),
                    )
                vv = v_t[t][:].rearrange("p (h c) -> p h c", h=2)
                nc.vector.tensor_copy(
                    vv[:, :, 0:64],
                    ps[:].rearrange("p (h c) -> p h c", h=2))

            def make_bg():
                # Generator of small PE work chunks, popped between
                # foreground score/attend matmuls.  Ordering constraints:
                # k ncol N before scores (unit 0, lc 0, mp >= 2N); q ncol N
                # before scores of (unit 0, lc N); v tile t of b0 before the
                # attend chunk reading it (block 2, pops precede attend
                # chunks within a slot); qk b1 before unit 2 scores; v b1
                # before unit 2 attends.
                for ncol in range(1, LC):
                    yield ('qk', 1, ncol)
                yield ('qk', 0, 1)
                for t in range(4, VT // 4):
                    yield ('v', t)
                yield ('qk', 0, 2)
                for t in range(VT // 4, VT // 2):
                    yield ('v', t)
                yield ('qk', 0, 3)
                for ncol in range(LC, 2 * LC):
                    yield ('qk', 1, ncol)
                    yield ('qk', 0, ncol)
                for t in range(VT // 2, VT):
                    yield ('v', t)

            bg = make_bg()
            bg_done = [False]
            bg_deficit = [0.0]  # us of background work we owe the stream
            emitted = {('qk', 1, 0), ('qk', 0, 0)}  # prologue

            def bg_step():
                item = next(bg, None)
                if item is None:
                    bg_done[0] = True
                    return False
                emitted.add(item)
                if item[0] == 'v':
                    emit_v_tile(item[1])
                    bg_deficit[0] -= 0.55
                else:
                    emit_qk_col(item[1], item[2])
                    bg_deficit[0] -= 1.9
                return True

            def bg_pop(budget_us):
                # emit background chunks worth ~budget_us of PE time
                bg_deficit[0] += budget_us
                while not bg_done[0] and bg_deficit[0] > 0:
                    if not bg_step():
                        return

            def bg_need(item):
                # structural deadline: force-drain the queue until `item`
                # has been emitted (it must precede its first consumer in
                # the in-order PE queue, or the kernel deadlocks).
                while item not in emitted and not bg_done[0]:
                    bg_step()

            # ---- attention unit pieces ----
            # one exchange per batch, fired after that batch's second
            # (hl=1) unit, carrying both head rows: [8, 128, 256]
            cc_in = [dram.tile([N_CORES, 128, 256], BF16, name=f"cc_in{i}")
                     for i in range(2)]
            cc_out = [dram.tile([N_CORES, 128, 256], BF16, name=f"cc_out{i}")
                      for i in range(2)]

            def emit_scores(hl, b, lc, mp):
                # one st tile = S^T for m-tiles (2mp, 2mp+1) x 512 l-cols
                ls = slice(b * L + lc * 512, b * L + (lc + 1) * 512)
                st = pst.tile([128, 1024], F32, tag="st")
                for h2 in range(2):
                    mt = 2 * mp + h2
                    nc.tensor.matmul(
                        st[:, h2 * 512:(h2 + 1) * 512],
                        kp_t[hl][:, b * L + mt * 128:b * L + (mt + 1) * 128],
                        qs_t[:, ls],
                        start=True, stop=True,
                    )
                pt = ptp.tile([128, 2, 512], BF16, tag="pt")
                nc.scalar.activation(pt[:], st[:], Exp, scale=SCALE)
                return pt

            def emit_av_chunk(hl, b, ov, pts, mt0, mt1):
                for mt in range(mt0, mt1):
                    nc.tensor.matmul(
                        ov[:],
                        v_t[b * MT + mt][:, hl * 128:(hl + 1) * 128],
                        pts[mt // 2][:, mt % 2, :],
                        start=(mt == 0), stop=(mt == MT - 1),
                    )

            def emit_norm(u, hl, b, lc, ov):
                # normalize ov rows 0:64 by its colsum row (64) and write
                # bf16 o^T.  The reciprocal row is broadcast across
                # partitions on the idle GpSimd engine, keeping the PE
                # queue out of the chain.  After the second (hl=1) unit of
                # a batch, both head rows of this window are final -- stage
                # the two 256-col a2a slots.
                sd = sdf[lc % 2]
                sf = scrf[lc % 2]
                bf = bcsf[lc % 2]
                nc.vector.tensor_copy(sd[0:1, :], ov[64:65, :])
                nc.vector.reciprocal_approx_fast(sf[0:1, :], sd[0:1, :])
                nc.gpsimd.partition_broadcast(bf[:], sf[0:1, :])
                hs = slice(hl * 64, (hl + 1) * 64)
                win = slice(b * L + lc * 512, b * L + (lc + 1) * 512)
                nc.vector.tensor_mul(oT_f[hs, win], ov[0:64, :], bf[:])
                if hl == 1:
                    for jj in range(2):
                        j = 2 * lc + jj
                        nc.sync.dma_start(
                            cc_in[b][j],
                            oT_f[:, b * L + j * 256:b * L + (j + 1) * 256])

            def emit_exchange(b):
                nc.gpsimd.collective_compute(
                    "AllToAll",
                    mybir.AluOpType.bypass,
                    ins=[cc_in[b].opt()],
                    outs=[cc_out[b].opt()],
                    replica_groups=[list(range(N_CORES))],
                )
                for k in range(N_CORES):
                    for h in range(2):
                        nc.sync.dma_start(
                            ogT_t[k][h * 64:(h + 1) * 64,
                                     b * 256:(b + 1) * 256],
                            cc_out[b][k][h * 64:(h + 1) * 64, :])

            # ---- emission schedule ----
            # Prologue: only k and q for ncol 0 -- the first scores matmul
            # needs just those, so the exp stream starts ~10us in; the
            # remaining k/q columns, all v tiles and batch-1 qkv are woven
            # between foreground matmuls via the background queue, each
            # ahead of its first consumer.
            emit_qk_col(1, 0)
            emit_qk_col(0, 0)
            for t in range(4):  # v tiles of cc0 -- same DMA dep, PE idle
                emitted.add(('v', t))
                emit_v_tile(t)

            # Blocks: one per (unit, lc).  Block i emits scores for chunk i
            # and (woven between them) the attend chain for chunk i-2 --
            # the lag keeps the PE queue from deadlocking on pt buffers and
            # gives the background queue room in the first two blocks.
            chunks = [(u, hl, b, lc)
                      for u, (hl, b) in enumerate(UNITS) for lc in range(LC)]
            pts_of = {}
            n_chunks = len(chunks)
            AVLAG = 2

            for i in range(n_chunks + AVLAG):
                sc = chunks[i] if i < n_chunks else None
                avi = i - AVLAG
                av = chunks[avi] if avi >= 0 else None
                if av is not None:
                    au, ahl, ab, alc = av
                    aov = pov.tile([128, 512], F32, tag="ov")
                    apts = pts_of.pop(avi)
                if sc is not None:
                    su, shl, sb, slc = sc
                    bg_need(('qk', 0, sb * LC + slc))  # q cols of this lc
                    pts = []
                    for mp in range(MT // 2):
                        bg_need(('qk', 1, sb * LC + mp // 2))  # k m-tiles
                        pts.append(emit_scores(shl, sb, slc, mp))
                        bg_pop(0.4 if av is not None else 1.0)
                    pts_of[i] = pts
                    if av is not None:
                        # solid 16-matmul attend chain after the scores: its
                        # exps finished a block ago, so it streams at full
                        # PE rate while the banked st tiles feed the exp
                        # stream.
                        bg_need(('v', ab * MT + MT - 1))
                        emit_av_chunk(ahl, ab, aov, apts, 0, MT)
                else:
                    # tail blocks: attend only (exp stream is draining)
                    bg_need(('v', ab * MT + MT - 1))
                    emit_av_chunk(ahl, ab, aov, apts, 0, MT)
                if av is not None:
                    emit_norm(au, ahl, ab, alc, aov)
                    if alc == LC - 1 and ahl == 1:
                        emit_exchange(ab)
            bg_pop(100)  # safety: drain any background leftovers

            # ---- output projection for our 512 rows ----
            # lt 0,1 = b0 (gated on exchanges 0,1 -- long arrived, runs
            # during the final exchange); lt 2,3 = b1 (gated on exchange 3).
            wout_t = [big.tile([128, D], BF16, tag=f"xT{k}", name=f"wout{k}")
                      for k in range(KT)]
            for k in range(KT):
                nc.sync.dma_start(wout_t[k][:],
                                  wout_ext[k * 128:(k + 1) * 128, :])
            for lt in range(4):
                for nt in range(2):
                    ps = pst.tile([128, 1024], F32, tag="st")
                    for k in range(KT):
                        nc.tensor.matmul(
                            ps[:, 0:512],
                            ogT_t[k][:, lt * 128:(lt + 1) * 128],
                            wout_t[k][:, nt * 512:(nt + 1) * 512],
                            start=(k == 0), stop=(k == KT - 1),
                        )
                    osb = small.tile([128, 512], BF16, tag="osb")
                    # alternate copy engines: ScalarE is idle at the tail
                    if nt == 0:
                        nc.vector.tensor_copy(osb[:], ps[:, 0:512])
                    else:
                        nc.scalar.copy(osb[:], ps[:, 0:512])
                    for h in range(2):
                        nc.sync.dma_start(
                            out_ext[lt * 128 + h * 64:lt * 128 + (h + 1) * 64,
                                    nt * 512:(nt + 1) * 512],
                            osb[h * 64:(h + 1) * 64, :])

    nc.compile()
    return nc


_NC_CACHE = None


def _get_nc():
    global _NC_CACHE
    if _NC_CACHE is None:
        _NC_CACHE = _build()
    return _NC_CACHE


def _make_in_maps(x, w_qkv, w_out):
    x = np.asarray(x, dtype=np.float32)
    w_qkv = np.asarray(w_qkv, dtype=np.float32)
    w_out = np.asarray(w_out, dtype=np.float32)
    bf = ml_dtypes.bfloat16
    xT = np.ascontiguousarray(
        x.transpose(2, 0, 1).reshape(D, BL)).astype(bf)
    wout_b = w_out.astype(bf)
    in_maps = []
    for c in range(N_CORES):
        cs = slice(c * 128, (c + 1) * 128)
        wqkv_c = np.ascontiguousarray(
            np.concatenate([w_qkv[:, cs], w_qkv[:, D:][:, cs],
                            w_qkv[:, 2 * D:][:, cs]], axis=1)
        ).astype(bf)
        in_maps.append({"xT": xT, "wqkv": wqkv_c, "wout": wout_b})
    return in_maps


def _run(x, w_qkv, w_out, trace=False):
    nc = _get_nc()
    in_maps = _make_in_maps(x, w_qkv, w_out)
    res = bass_utils.run_bass_kernel_spmd(
        nc, in_maps, list(range(N_CORES)), trace=trace)
    out = np.empty((B, L, D), dtype=np.float32)
    for c in range(N_CORES):
        r = np.asarray(res.results[c]["out"]).astype(np.float32)
        out[0, c * 256:(c + 1) * 256, :] = r[0:256]
        out[1, c * 256:(c + 1) * 256, :] = r[256:512]
    return out, res


def kernel(x, w_qkv, w_out):
    out, _ = _run(x, w_qkv, w_out, trace=False)
    return out


# revision 57
# speedup vs baseline: 1.0450x; 1.0002x over previous
"""Multi-head attention on 8 TRN2 NeuronCores.

Problem: x[2, 2048, 1024], w_qkv[1024, 3072], w_out[1024, 1024] (f32).
  qkv = x @ w_qkv; q,k,v per 16 heads of dim 64; softmax(q k^T / 8) v; out proj.

Sharding: 16 heads split 8 ways (one head-PAIR per core, both batches on
every core).  Output ownership is interleaved by batch: core c owns rows
(b=0, cols c*256:(c+1)*256) and (b=1, same cols), so that after each
(head, batch) attention unit a small AllToAll of [8, 64, 256] can fire in
which EVERY slot carries final data.  Four such exchanges; only the last
is exposed, and it overlaps the b=0 half of the output projection.

The kernel is scheduled around the Scalar engine (exp is 140us of the
~190us total work): the k-projection for batch 0 is emitted first so the
first scores matmul (and exp) issues at ~14us, and every other piece of
PE work (q cols, v tiles, batch-1 qkv, broadcast matmuls) is woven
between score/attend matmuls so the exp stream never stalls on a block
of projection work.

Layout: scores are computed TRANSPOSED (S^T[m, l] tiles); softmax sum
over the partition axis comes free from a ones-column appended to v in
the attn@v matmul (out rows = [o^T; colsums]).  exp() runs on ScalarE
straight out of PSUM with the 1/8 scale folded in.  Normalization:
colsums row -> SBUF, reciprocal_approx_fast, GpSimd partition_broadcast
across 64 partitions, one DVE multiply writes normalized o^T bf16 --
no transposes, and nothing in the chain touches the PE or ScalarE.

q is stored STACKED ([q_h0; q_h1] on 128 partitions, no zero pad); the
per-head k tiles are zero-padded in complementary halves (k_h0 on rows
0:64, k_h1 on rows 64:128) so each scores matmul contracts over the full
128 partitions (keeps the HAM clock gate at 2.4 GHz) while the other
head's q rows are killed by zero weights.

Compute dtype bf16 (f32 accumulation in PSUM).
"""

import sys
import types

sys.path.insert(0, "/opt/trn_rl_repo")

import numpy as np
import ml_dtypes

import concourse.bass as bass
import concourse.mybir as mybir
import concourse.tile as tile
from concourse import bacc
from concourse import bass_utils
from concourse.masks import make_identity

# If the image's antenv lacks the axon_hooks module, run_bass_kernel_spmd's
# trace path (reachable via BASS_TRACE=1) would die on import.  Provide the
# registry so tracing degrades gracefully instead (hook stays None unless
# trn_boot registered one).
try:
    import antenv.axon_hooks  # noqa: F401
except ImportError:
    _hooks = types.ModuleType("antenv.axon_hooks")
    _hooks._hook = None
    _hooks.set_axon_ntff_profile_hook = (
        lambda h: setattr(_hooks, "_hook", h))
    _hooks.get_axon_ntff_profile_hook = lambda: _hooks._hook
    sys.modules["antenv.axon_hooks"] = _hooks

# Artifact upload needs bucket credentials; fall back to the local dir so a
# traced run in a sandboxed container still completes.
_orig_upload = bass_utils.upload_artifacts


def _safe_upload(tmpdir):
    try:
        return _orig_upload(tmpdir)
    except Exception:
        return tmpdir


bass_utils.upload_artifacts = _safe_upload

B, L, D, H, DH = 2, 2048, 1024, 16, 64
BL = B * L  # 4096
SCALE = DH ** -0.5
N_CORES = 8
BF16 = mybir.dt.bfloat16
F32 = mybir.dt.float32
Exp = mybir.ActivationFunctionType.Exp

KT = D // 128          # 8 k-tiles over the model dim
MT = L // 128          # 16 m-tiles per batch
LC = L // 512          # 4 l-chunks of 512 per batch
VT = BL // 128         # 32 v row-tiles over (b, l)
# units in emission order: (hl, b)
UNITS = [(0, 0), (1, 0), (0, 1), (1, 1)]


def _build():
    nc = bacc.Bacc("TRN2", target_bir_lowering=False, debug=False,
                   num_devices=N_CORES)
    xT_ext = nc.declare_dram_parameter("xT", [D, BL], BF16, isOutput=False)
    # q (128) | k (128) | v (128) columns for this core's head pair
    wqkv_ext = nc.declare_dram_parameter("wqkv", [D, 384], BF16,
                                         isOutput=False)
    wout_ext = nc.declare_dram_parameter("wout", [D, D], BF16, isOutput=False)
    out_ext = nc.declare_dram_parameter("out", [512, D], BF16, isOutput=True)

    with tile.TileContext(nc) as tc:
        with (
            tc.tile_pool(name="big", bufs=1) as big,
            tc.tile_pool(name="pt", bufs=24) as ptp,
            tc.tile_pool(name="small", bufs=3) as small,
            tc.tile_pool(name="psum_st", bufs=3, space="PSUM") as pst,
            tc.tile_pool(name="psum_ov", bufs=2, space="PSUM") as pov,
            tc.tile_pool(name="dram", bufs=1, space="DRAM") as dram,
        ):
            # ---- static SBUF tensors ----
            xT_t = [big.tile([128, BL], BF16, tag=f"xT{k}", name=f"xT{k}")
                    for k in range(KT)]
            wqkv_t = [big.tile([128, 384], BF16, tag=f"wqkv{k}",
                               name=f"wqkv{k}") for k in range(KT)]
            # DMA priority: the 8 wqkv tiles land on queues 0-7 while the
            # first xT b0 column-chunks land on 8-15 in parallel; b0 moves
            # in fine [128, 512] chunks (lowest latency to first scores),
            # b1 in coarser [128, 2048] chunks (fewer descriptors).
            for k in range(KT):
                nc.sync.dma_start(wqkv_t[k][:],
                                  wqkv_ext[k * 128:(k + 1) * 128, :])
            for cc in range(4):
                for k in range(KT):
                    cs = slice(cc * 512, (cc + 1) * 512)
                    nc.sync.dma_start(
                        xT_t[k][:, cs],
                        xT_ext[k * 128:(k + 1) * 128, cs])
            for k in range(KT):
                nc.sync.dma_start(
                    xT_t[k][:, 2048:4096],
                    xT_ext[k * 128:(k + 1) * 128, 2048:4096])

            ident_b = big.tile([128, 128], BF16, tag="ident_b")
            make_identity(nc, ident_b[:])

            # Preload the Exp activation table while DMAs stream in, so the
            # first real exp doesn't pay the ~1.3us table load.  Emitted
            # before the other Pool-engine memsets so it lands early.
            dum = big.tile([1, 16], F32, tag="dum")
            nc.gpsimd.memset(dum[:], 0.0)
            dum2 = big.tile([1, 16], BF16, tag="dum2")
            nc.scalar.activation(dum2[:], dum[:], Exp, scale=1.0)

            # q stacked (h0 rows 0:64, h1 rows 64:128) -- no padding.
            qs_t = big.tile([128, BL], BF16, tag="qs", name="qs")
            # k per head, zero-padded in complementary halves.
            kp_t = [big.tile([128, BL], BF16, tag=f"kp{h}", name=f"kp{h}")
                    for h in range(2)]
            nc.vector.memset(kp_t[0][64:128, :], 0.0)
            nc.vector.memset(kp_t[1][0:64, :], 0.0)
            # v: cols [h*128 : h*128+64] = head h, +64 = ones, rest zero
            v_t = [big.tile([128, 256], BF16, tag=f"v{t}", name=f"v{t}")
                   for t in range(VT)]
            for t in range(VT):
                vv = v_t[t][:].rearrange("p (h c) -> p h c", h=2)
                nc.gpsimd.memset(vv[:, :, 64:65], 1.0)
                nc.gpsimd.memset(vv[:, :, 65:128], 0.0)
            # final o^T for our 2 heads, all 4096 cols
            oT_f = big.tile([128, BL], BF16, tag="oT")
            # received heads for our 512 output rows (cols 0:256 b0, 256:512 b1)
            ogT_t = [big.tile([128, 512], BF16, tag=f"ogT{k}", name=f"ogT{k}")
                     for k in range(KT)]
            # softmax-denominator staging: denom row copied out of PSUM
            # (reciprocal_approx_fast misreads PSUM inputs), its reciprocal,
            # and the partition-broadcast copy; double-buffered by lc parity.
            sdf = [big.tile([1, 512], F32, tag=f"sdf{i}", name=f"sdf{i}")
                   for i in range(2)]
            scrf = [big.tile([1, 512], F32, tag=f"scrf{i}", name=f"scrf{i}")
                    for i in range(2)]
            bcsf = [big.tile([64, 512], F32, tag=f"bcsf{i}", name=f"bcsf{i}")
                    for i in range(2)]

            # Warm the HAM clock gate during the initial DMA wait: ~30
            # back-to-back matmuls lift the PE to 2.4 GHz before the first
            # real matmul issues.  Output is scratch nobody reads.
            warm = pov.tile([128, 512], F32, tag="ov", name="warm")
            for i in range(30):
                nc.tensor.matmul(warm[:, 0:128], ident_b[:], ident_b[:],
                                 start=(i == 0), stop=(i == 29))

            # ---- background PE work generators (woven between scores) ----
            def emit_qk_col(m, ncol):
                # m: 0 -> q, 1 -> k.  One 512-col chunk of the projection.
                ps = pst.tile([128, 512], F32, tag="st",
                              name=f"qk{m}_{ncol}")
                for k in range(KT):
                    nc.tensor.matmul(
                        ps[:],
                        wqkv_t[k][:, m * 128:(m + 1) * 128],
                        xT_t[k][:, ncol * 512:(ncol + 1) * 512],
                        start=(k == 0), stop=(k == KT - 1),
                    )
                cs = slice(ncol * 512, (ncol + 1) * 512)
                if m == 0:
                    nc.vector.tensor_copy(qs_t[:, cs], ps[:])
                else:
                    nc.vector.tensor_copy(kp_t[0][0:64, cs], ps[0:64, :])
                    nc.vector.tensor_copy(kp_t[1][64:128, cs],
                                          ps[64:128, :])

            def emit_v_tile(t):
                ps = pst.tile([128, 128], F32, tag="st", name=f"v_ps{t}")
                for k in range(KT):
                    nc.tensor.matmul(
                        ps[:],
                        xT_t[k][:, t * 128:(t + 1) * 128],
                        wqkv_t[k][:, 256:384],
                        start=(k == 0), stop=(k == KT - 1),
                    )
                vv = v_t[t][:].rearrange("p (h c) -> p h c", h=2)
                nc.vector.tensor_copy(
                    vv[:, :, 0:64],
                    ps[:].rearrange("p (h c) -> p h c", h=2))

            def make_bg():
                # Generator of small PE work chunks, popped between
                # foreground score/attend matmuls.  Ordering constraints:
                # k ncol N before scores (unit 0, lc 0, mp >= 2N); q ncol N
                # before scores of (unit 0, lc N); v tile t of b0 before the
                # attend chunk reading it (block 2, pops precede attend
                # chunks within a slot); qk b1 before unit 2 scores; v b1
                # before unit 2 attends.
                for ncol in range(1, LC):
                    yield ('qk', 1, ncol)
                yield ('qk', 0, 1)
                for t in range(4, VT // 4):
                    yield ('v', t)
                yield ('qk', 0, 2)
                for t in range(VT // 4, VT // 2):
                    yield ('v', t)
                yield ('qk', 0, 3)
                for ncol in range(LC, 2 * LC):
                    yield ('qk', 1, ncol)
                    yield ('qk', 0, ncol)
                for t in range(VT // 2, VT):
                    yield ('v', t)

            bg = make_bg()
            bg_done = [False]
            bg_deficit = [0.0]  # us of background work we owe the stream
            emitted = {('qk', 1, 0), ('qk', 0, 0)}  # prologue

            def bg_step():
                item = next(bg, None)
                if item is None:
                    bg_done[0] = True
                    return False
                emitted.add(item)
                if item[0] == 'v':
                    emit_v_tile(item[1])
                    bg_deficit[0] -= 0.55
                else:
                    emit_qk_col(item[1], item[2])
                    bg_deficit[0] -= 1.9
                return True

            def bg_pop(budget_us):
                # emit background chunks worth ~budget_us of PE time
                bg_deficit[0] += budget_us
                while not bg_done[0] and bg_deficit[0] > 0:
                    if not bg_step():
                        return

            def bg_need(item):
                # structural deadline: force-drain the queue until `item`
                # has been emitted (it must precede its first consumer in
                # the in-order PE queue, or the kernel deadlocks).
                while item not in emitted and not bg_done[0]:
                    bg_step()

            # ---- attention unit pieces ----
            # one exchange per batch, fired after that batch's second
            # (hl=1) unit, carrying both head rows: [8, 128, 256]
            cc_in = [dram.tile([N_CORES, 128, 256], BF16, name=f"cc_in{i}")
                     for i in range(2)]
            cc_out = [dram.tile([N_CORES, 128, 256], BF16, name=f"cc_out{i}")
                      for i in range(2)]

            def emit_scores(hl, b, lc, mp):
                # one st tile = S^T for m-tiles (2mp, 2mp+1) x 512 l-cols
                ls = slice(b * L + lc * 512, b * L + (lc + 1) * 512)
                st = pst.tile([128, 1024], F32, tag="st")
                for h2 in range(2):
                    mt = 2 * mp + h2
                    nc.tensor.matmul(
                        st[:, h2 * 512:(h2 + 1) * 512],
                        kp_t[hl][:, b * L + mt * 128:b * L + (mt + 1) * 128],
                        qs_t[:, ls],
                        start=True, stop=True,
                    )
                pt = ptp.tile([128, 2, 512], BF16, tag="pt")
                nc.scalar.activation(pt[:], st[:], Exp, scale=SCALE)
                return pt

            def emit_av_chunk(hl, b, ov, pts, mt0, mt1):
                for mt in range(mt0, mt1):
                    nc.tensor.matmul(
                        ov[:],
                        v_t[b * MT + mt][:, hl * 128:(hl + 1) * 128],
                        pts[mt // 2][:, mt % 2, :],
                        start=(mt == 0), stop=(mt == MT - 1),
                    )

            def emit_norm(u, hl, b, lc, ov):
                # normalize ov rows 0:64 by its colsum row (64) and write
                # bf16 o^T.  The reciprocal row is broadcast across
                # partitions on the idle GpSimd engine, keeping the PE
                # queue out of the chain.  After the second (hl=1) unit of
                # a batch, both head rows of this window are final -- stage
                # the two 256-col a2a slots.
                sd = sdf[lc % 2]
                sf = scrf[lc % 2]
                bf = bcsf[lc % 2]
                nc.vector.tensor_copy(sd[0:1, :], ov[64:65, :])
                nc.vector.reciprocal_approx_fast(sf[0:1, :], sd[0:1, :])
                nc.gpsimd.partition_broadcast(bf[:], sf[0:1, :])
                hs = slice(hl * 64, (hl + 1) * 64)
                win = slice(b * L + lc * 512, b * L + (lc + 1) * 512)
                nc.vector.tensor_mul(oT_f[hs, win], ov[0:64, :], bf[:])
                if hl == 1:
                    for jj in range(2):
                        j = 2 * lc + jj
                        nc.sync.dma_start(
                            cc_in[b][j],
                            oT_f[:, b * L + j * 256:b * L + (j + 1) * 256])

            def emit_exchange(b):
                nc.gpsimd.collective_compute(
                    "AllToAll",
                    mybir.AluOpType.bypass,
                    ins=[cc_in[b].opt()],
                    outs=[cc_out[b].opt()],
                    replica_groups=[list(range(N_CORES))],
                )
                for k in range(N_CORES):
                    for h in range(2):
                        nc.sync.dma_start(
                            ogT_t[k][h * 64:(h + 1) * 64,
                                     b * 256:(b + 1) * 256],
                            cc_out[b][k][h * 64:(h + 1) * 64, :])

            # ---- emission schedule ----
            # Prologue: only k and q for ncol 0 -- the first scores matmul
            # needs just those, so the exp stream starts ~10us in; the
            # remaining k/q columns, all v tiles and batch-1 qkv are woven
            # between foreground matmuls via the background queue, each
            # ahead of its first consumer.
            emit_qk_col(1, 0)
            emit_qk_col(0, 0)
            for t in range(4):  # v tiles of cc0 -- same DMA dep, PE idle
                emitted.add(('v', t))
                emit_v_tile(t)

            # Blocks: one per (unit, lc).  Block i emits scores for chunk i
            # and (woven between them) the attend chain for chunk i-2 --
            # the lag keeps the PE queue from deadlocking on pt buffers and
            # gives the background queue room in the first two blocks.
            chunks = [(u, hl, b, lc)
                      for u, (hl, b) in enumerate(UNITS) for lc in range(LC)]
            pts_of = {}
            n_chunks = len(chunks)
            AVLAG = 2

            for i in range(n_chunks + AVLAG):
                sc = chunks[i] if i < n_chunks else None
                avi = i - AVLAG
                av = chunks[avi] if avi >= 0 else None
                if av is not None:
                    au, ahl, ab, alc = av
                    aov = pov.tile([128, 512], F32, tag="ov")
                    apts = pts_of.pop(avi)
                if sc is not None:
                    su, shl, sb, slc = sc
                    bg_need(('qk', 0, sb * LC + slc))  # q cols of this lc
                    pts = []
                    for mp in range(MT // 2):
                        bg_need(('qk', 1, sb * LC + mp // 2))  # k m-tiles
                        pts.append(emit_scores(shl, sb, slc, mp))
                        bg_pop(0.4 if av is not None else 1.0)
                    pts_of[i] = pts
                    if av is not None:
                        # solid 16-matmul attend chain after the scores: its
                        # exps finished a block ago, so it streams at full
                        # PE rate while the banked st tiles feed the exp
                        # stream.
                        bg_need(('v', ab * MT + MT - 1))
                        emit_av_chunk(ahl, ab, aov, apts, 0, MT)
                else:
                    # tail blocks: attend only (exp stream is draining)
                    bg_need(('v', ab * MT + MT - 1))
                    emit_av_chunk(ahl, ab, aov, apts, 0, MT)
                if av is not None:
                    emit_norm(au, ahl, ab, alc, aov)
                    if alc == LC - 1 and ahl == 1:
                        emit_exchange(ab)
            bg_pop(100)  # safety: drain any background leftovers

            # ---- output projection for our 512 rows ----
            # lt 0,1 = b0 (gated on exchanges 0,1 -- long arrived, runs
            # during the final exchange); lt 2,3 = b1 (gated on exchange 3).
            wout_t = [big.tile([128, D], BF16, tag=f"xT{k}", name=f"wout{k}")
                      for k in range(KT)]
            for k in range(KT):
                nc.sync.dma_start(wout_t[k][:],
                                  wout_ext[k * 128:(k + 1) * 128, :])
            for lt in range(4):
                for nt in range(2):
                    ps = pst.tile([128, 1024], F32, tag="st")
                    for k in range(KT):
                        nc.tensor.matmul(
                            ps[:, 0:512],
                            ogT_t[k][:, lt * 128:(lt + 1) * 128],
                            wout_t[k][:, nt * 512:(nt + 1) * 512],
                            start=(k == 0), stop=(k == KT - 1),
                        )
                    osb = small.tile([128, 512], BF16, tag="osb")
                    # alternate copy engines: ScalarE is idle at the tail
                    if nt == 0:
                        nc.vector.tensor_copy(osb[:], ps[:, 0:512])
                    else:
                        nc.scalar.copy(osb[:], ps[:, 0:512])
                    for h in range(2):
                        nc.sync.dma_start(
                            out_ext[lt * 128 + h * 64:lt * 128 + (h + 1) * 64,
                                    nt * 512:(nt + 1) * 512],
                            osb[h * 64:(h + 1) * 64, :])

    nc.compile()
    return nc


_NC_CACHE = None


def _get_nc():
    global _NC_CACHE
    if _NC_CACHE is None:
        _NC_CACHE = _build()
    return _NC_CACHE


def _make_in_maps(x, w_qkv, w_out):
    x = np.asarray(x, dtype=np.float32)
    w_qkv = np.asarray(w_qkv, dtype=np.float32)
    w_out = np.asarray(w_out, dtype=np.float32)
    bf = ml_dtypes.bfloat16
    xT = np.ascontiguousarray(
        x.transpose(2, 0, 1).reshape(D, BL)).astype(bf)
    wout_b = w_out.astype(bf)
    in_maps = []
    for c in range(N_CORES):
        cs = slice(c * 128, (c + 1) * 128)
        wqkv_c = np.ascontiguousarray(
            np.concatenate([w_qkv[:, cs], w_qkv[:, D:][:, cs],
                            w_qkv[:, 2 * D:][:, cs]], axis=1)
        ).astype(bf)
        in_maps.append({"xT": xT, "wqkv": wqkv_c, "wout": wout_b})
    return in_maps


def _run(x, w_qkv, w_out, trace=False):
    nc = _get_nc()
    in_maps = _make_in_maps(x, w_qkv, w_out)
    res = bass_utils.run_bass_kernel_spmd(
        nc, in_maps, list(range(N_CORES)), trace=trace)
    out = np.empty((B, L, D), dtype=np.float32)
    for c in range(N_CORES):
        r = np.asarray(res.results[c]["out"]).astype(np.float32)
        out[0, c * 256:(c + 1) * 256, :] = r[0:256]
        out[1, c * 256:(c + 1) * 256, :] = r[256:512]
    return out, res


def kernel(x, w_qkv, w_out):
    out, _ = _run(x, w_qkv, w_out, trace=False)
    return out
